# revision 1
# baseline (speedup 1.0000x reference)
"""KAT rational-group activation kernel for Trainium2 (Bass/Tile), 8-core SPMD.

Computes out = num(x) / den(x) elementwise over x:(4,4096,2048) f32, where
  num(x) = Horner(x, a0..a5)            (numerator coeffs shared everywhere)
  den(x) = Horner(x, [1, |b1..b4|])     (per-group g = channel // 256)

Strategy: shard the sequence dim L across 8 NeuronCores (pure data parallel).
Per core, tiles of [128 positions, 2048 channels] f32 stream through 5 DVE
instructions per tile:
  1. KAT_DEN   (custom, per-group free-dim slice, exact reference Horner order)
  2. reciprocal_approx_fast (stock custom op, ~51 ULP)
  3. KAT_NUMQ  (custom, Horner prefix through a2)
  4. KAT_NUMM  (custom, Horner finish through a0)
  5. tensor_mul (num * recip)
Coefficients are baked as compile-time instruction immediates (3 per op) plus
one [P,1] spilled scalar rides in1 from a tiny replicated coef tensor.
"""

import numpy as np

B, L, D = 4, 4096, 2048
N_CORES = 8
L_SH = L // N_CORES            # 512
ROWS = B * L_SH                # 2048 rows per core shard
P = 128                        # SBUF partitions
N_TILES = ROWS // P            # 16 tiles of [128, D]

_OPS_CACHE = {}


def _register_ops():
    """Define + register the three KAT custom DVE ops (idempotent)."""
    if _OPS_CACHE:
        return _OPS_CACHE

    from concourse import dve_ops
    from concourse.dve_ops import DveOp
    from concourse.dve_spec import (
        C0, C1, C2, C3, One, Spec, Src0, Src1,
        _has_src1, _spill_c3_to_src1, lower,
    )
    from concourse.dve_uop import DveOpSpec

    # den = (((c4*x + c3)*x + c2)*x + c1)*x + 1   [C0..C2 imm, C3 -> in1 spill]
    den_body = _spill_c3_to_src1(
        (((C0 * Src0 + C1) * Src0 + C2) * Src0 + C3) * Src0 + One
    )
    den_ref = lambda in0, in1, s0, s1, imm2: (
        (((s0 * in0.astype(np.float32) + s1) * in0 + imm2) * in0
         + np.asarray(in1, np.float32).reshape(-1, 1)) * in0 + 1.0
    )

    # Q = ((a5*x + a4)*x + a3)*x + a2             [C0..C2 imm, C3 -> in1 spill]
    numq_body = _spill_c3_to_src1(
        ((C0 * Src0 + C1) * Src0 + C2) * Src0 + C3
    )
    numq_ref = lambda in0, in1, s0, s1, imm2: (
        ((s0 * in0.astype(np.float32) + s1) * in0 + imm2) * in0
        + np.asarray(in1, np.float32).reshape(-1, 1)
    )

    # M = (Q*x + a1)*x + a0                        [two full streams]
    numm_body = (Src0 * Src1 + C0) * Src1 + C1
    numm_ref = lambda in0, in1, s0, s1, imm2: (
        (in0.astype(np.float32) * in1 + s0) * in1 + s1
    )

    # M1 = (A' + ka)*(B' + kb)    [factored-numerator quadratic join]
    fac1_body = (Src0 + C0) * (Src1 + C1)
    fac1_ref = lambda in0, in1, s0, s1, imm2: (
        (in0.astype(np.float32) + s0) * (in1 + s1)
    )

    defs = [
        ("KAT_DEN", den_body, den_ref),
        ("KAT_NUMQ", numq_body, numq_ref),
        ("KAT_NUMM", numm_body, numm_ref),
        ("KAT_FAC1", fac1_body, fac1_ref),
    ]

    existing = {op.name for op in dve_ops.OPS}
    for i, (name, body, ref) in enumerate(defs):
        if name in existing:
            _OPS_CACHE[name] = next(op for op in dve_ops.OPS if op.name == name)
            continue
        spec = Spec(body=body, reference=ref)
        row = max(dve_ops._SUB_OPCODE_FOR_NAME.values()) + 1
        assert row < 0x20, "custom DVE row field overflow"
        dve_ops._SUB_OPCODE_FOR_NAME[name] = row
        shas = {}
        for ver in ("v3", "v4"):
            uops = lower(spec, ver=ver)
            shas[ver] = DveOpSpec(
                name=name, opcode=row, uops=uops, rd1_en=_has_src1(spec)
            ).sha(ver)
        op = DveOp(name, spec, subdim=False, uops_sha=shas)
        dve_ops.OPS.append(op)
        dve_ops.CUSTOM_DVE_SPECS[name] = spec
        _OPS_CACHE[name] = op
    return _OPS_CACHE


VARIANT = "opt"  # one of: "dve", "gpsimd_mul", "act_recip", "gpsimd_div", "opt"


def _build_module(a, c, G, variant=None):
    """Trace the per-core Bass module. a:(6,) numerator, c:(G,5) |den| coeffs."""
    import concourse.bacc as bacc
    import concourse.mybir as mybir
    from concourse.tile import TileContext

    variant = VARIANT if variant is None else variant
    ops = _register_ops()
    f32 = mybir.dt.float32
    W = D // G  # channels per group

    nc = bacc.Bacc("TRN2", target_bir_lowering=False)
    x = nc.dram_tensor("x", (ROWS, D), f32, kind="ExternalInput")
    coef = nc.dram_tensor("coef", (P, G + 4), f32, kind="ExternalInput")
    y = nc.dram_tensor("y", (ROWS, D), f32, kind="ExternalOutput")

    if variant == "opt":
        return _build_opt(nc, x, coef, y, a, c, G, ops, f32, cfg=globals().get('_OPT_CFG_OVERRIDE'))
    if variant == "mix":
        fac = _factor_numerator(a)
        if fac is None:
            return _build_opt(nc, x, coef, y, a, c, G, ops, f32)
        return _build_mix(nc, x, coef, y, a, c, G, ops, f32, fac,
                          n_b=int(globals().get('_MIX_NB', 7)))

    with TileContext(nc) as tc:
        with tc.tile_pool(name="const", bufs=1) as cpool, \
             tc.tile_pool(name="work", bufs=3) as pool:
            ct = cpool.tile([P, G + 4], f32)
            nc.sync.dma_start(out=ct[:], in_=coef[:, :])
            for i in range(N_TILES):
                r0 = i * P
                xt = pool.tile([P, D], f32, tag="x")
                nc.sync.dma_start(out=xt[:], in_=x[r0:r0 + P, :])

                dent = pool.tile([P, D], f32, tag="den")
                for g in range(G):
                    sl = slice(g * W, (g + 1) * W)
                    nc.vector._custom_dve(
                        ops["KAT_DEN"],
                        out=dent[:, sl], in0=xt[:, sl], in1=ct[:, g:g + 1],
                        s0=float(c[g, 4]), s1=float(c[g, 3]), imm2=float(c[g, 2]),
                    )
                if variant != "gpsimd_div":
                    rt = pool.tile([P, D], f32, tag="r")
                    if variant == "act_recip":
                        imm = lambda v: mybir.ImmediateValue(
                            dtype=mybir.dt.float32, value=v
                        )
                        nc.scalar.add_instruction(
                            mybir.InstActivation(
                                name=nc.get_next_instruction_name(),
                                func=mybir.ActivationFunctionType.Reciprocal,
                                ins=[nc.scalar.lower_ap(dent[:]),
                                     imm(0.0), imm(1.0), imm(0.0)],
                                outs=[nc.scalar.lower_ap(rt[:])],
                            )
                        )
                    else:
                        nc.vector.reciprocal_approx_fast(out=rt[:], in_=dent[:])

                qt = pool.tile([P, D], f32, tag="q")
                nc.vector._custom_dve(
                    ops["KAT_NUMQ"],
                    out=qt[:], in0=xt[:], in1=ct[:, G:G + 1],
                    s0=float(a[5]), s1=float(a[4]), imm2=float(a[3]),
                )
                mt = pool.tile([P, D], f32, tag="m")
                nc.vector._custom_dve(
                    ops["KAT_NUMM"],
                    out=mt[:], in0=qt[:], in1=xt[:],
                    s0=float(a[1]), s1=float(a[0]),
                )
                ot = pool.tile([P, D], f32, tag="o")
                if variant == "dve":
                    nc.vector.tensor_mul(ot[:], mt[:], rt[:])
                elif variant == "gpsimd_div":
                    nc.gpsimd.tensor_tensor(
                        ot[:], mt[:], dent[:], mybir.AluOpType.divide
                    )
                else:
                    nc.gpsimd.tensor_mul(ot[:], mt[:], rt[:])
                nc.sync.dma_start(out=y[r0:r0 + P, :], in_=ot[:])
    nc.compile()
    return nc


OPT_CFG = dict(S=1, reuse_m=False, reuse_o=True, bufs_x=4, bufs_work=3)


def _build_opt(nc, x, coef, y, a, c, G, ops, f32, cfg=None):
    """act_recip + gpsimd_mul + [128, S*D] multi-row tiles + tile reuse.

    Each tile holds S row-blocks: tile[p, s*D + ch] = x[r0 + s*P + p, ch].
    Per-group den runs on 3D APs [P, S, W]; Q/M/recip/mul on the full tile.
    M reuses den's tile, out reuses x's tile (WAR handled by Tile deps).
    """
    import concourse.mybir as mybir
    from concourse.tile import TileContext

    cfg = {**OPT_CFG, **(cfg or {})}
    S = cfg["S"]
    FD = S * D
    W = D // G
    n_big = ROWS // (P * S)

    imm = lambda v: mybir.ImmediateValue(dtype=mybir.dt.float32, value=v)

    with TileContext(nc) as tc:
        with tc.tile_pool(name="const", bufs=1) as cpool, \
             tc.tile_pool(name="xo", bufs=cfg["bufs_x"]) as xpool, \
             tc.tile_pool(name="work", bufs=cfg["bufs_work"]) as pool:
            ct = cpool.tile([P, G + 4], f32)
            nc.sync.dma_start(out=ct[:], in_=coef[:, :])
            for i in range(n_big):
                r0 = i * P * S
                xt = xpool.tile([P, FD], f32, tag="x")
                x3 = xt[:].rearrange("p (s c) -> p s c", s=S)
                xsrc = x[r0:r0 + P * S, :].rearrange("(s p) c -> p s c", s=S)
                nc.sync.dma_start(out=x3, in_=xsrc)
                dent = pool.tile([P, FD], f32, tag="den")
                d3 = dent[:].rearrange("p (s c) -> p s c", s=S)
                for g in range(G):
                    nc.vector._custom_dve(
                        ops["KAT_DEN"],
                        out=d3[:, :, g * W:(g + 1) * W],
                        in0=x3[:, :, g * W:(g + 1) * W],
                        in1=ct[:, g:g + 1],
                        s0=float(c[g, 4]), s1=float(c[g, 3]), imm2=float(c[g, 2]),
                    )
                rt = pool.tile([P, FD], f32, tag="r")
                nc.scalar.add_instruction(
                    mybir.InstActivation(
                        name=nc.get_next_instruction_name(),
                        func=mybir.ActivationFunctionType.Reciprocal,
                        ins=[nc.scalar.lower_ap(dent[:]),
                             imm(0.0), imm(1.0), imm(0.0)],
                        outs=[nc.scalar.lower_ap(rt[:])],
                    )
                )
                qt = pool.tile([P, FD], f32, tag="q")
                nc.vector._custom_dve(
                    ops["KAT_NUMQ"],
                    out=qt[:], in0=xt[:], in1=ct[:, G:G + 1],
                    s0=float(a[5]), s1=float(a[4]), imm2=float(a[3]),
                )
                mt = dent if cfg["reuse_m"] else pool.tile([P, FD], f32, tag="m")
                nc.vector._custom_dve(
                    ops["KAT_NUMM"],
                    out=mt[:], in0=qt[:], in1=xt[:],
                    s0=float(a[1]), s1=float(a[0]),
                )
                ot = xt if cfg["reuse_o"] else pool.tile([P, FD], f32, tag="o")
                nc.gpsimd.tensor_mul(ot[:], mt[:], rt[:])
                ydst = y[r0:r0 + P * S, :].rearrange("(s p) c -> p s c", s=S)
                o3 = ot[:].rearrange("p (s c) -> p s c", s=S)
                nc.sync.dma_start(out=ydst, in_=o3)
    nc.compile()
    return nc


def _factor_numerator(a):
    """num = a5(x-e)(x^2+p1x+q1)(x^2+p2x+q2) -> ACT-Square form, or None.

    Returns (e, (h1, k1), (h2, k2)) with quadratic x^2+px+q = (x+h)^2 + k,
    h = p/2, k = q - p^2/4. Validates factored fp32 eval against fp64 Horner
    on the relevant input range; None on degeneracy or excessive error.
    """
    a = np.asarray(a, np.float64)
    if abs(a[5]) < 1e-20 * max(1.0, np.abs(a).max()):
        return None
    r = np.roots(a[::-1])                       # roots of sum a_k x^k
    reals = sorted([z.real for z in r if abs(z.imag) < 1e-9])
    pairs = []
    used = np.zeros(len(r), bool)
    for i, z in enumerate(r):
        if used[i] or abs(z.imag) < 1e-9:
            continue
        for j, w in enumerate(r):
            if j > i and not used[j] and abs(z.conjugate() - w) < 1e-6 * max(1, abs(z)):
                pairs.append((z, w)); used[i] = used[j] = True
                break
    real_roots = [z.real for i, z in enumerate(r) if not used[i] and abs(z.imag) < 1e-9]
    if len(real_roots) % 2 == 0:
        return None                              # quintic must leave odd count
    e = min(real_roots, key=abs)                 # linear factor: smallest root
    real_roots.remove(e)
    quads = [(-(z + w).real, (z * w).real) for z, w in pairs]
    while real_roots:
        u = real_roots.pop(); v = real_roots.pop()
        quads.append((-(u + v), u * v))
    if len(quads) != 2:
        return None
    (p1, q1), (p2, q2) = quads
    h1, k1 = p1 / 2, q1 - p1 * p1 / 4
    h2, k2 = p2 / 2, q2 - p2 * p2 / 4
    # fp32 fidelity check vs fp64 Horner on the data range
    xs = np.linspace(-6, 6, 20001)
    exact = np.polyval(a[::-1], xs)
    x32 = xs.astype(np.float32)
    A = (x32 + np.float32(h1)) ** 2 + np.float32(k1)
    Bq = (x32 + np.float32(h2)) ** 2 + np.float32(k2)
    lin = np.float32(a[5]) * x32 - np.float32(a[5] * e)
    fac = (A * Bq * lin).astype(np.float64)
    scale = np.abs(exact).max()
    if np.abs(fac - exact).max() > 2e-6 * scale:
        return None
    return float(e), (float(h1), float(k1)), (float(h2), float(k2))


def _build_mix(nc, x, coef, y, a, c, G, ops, f32, fac, n_b):
    """Plan-A/Plan-B mixed tiles. Plan B (n_b of 16 tiles): numerator via two
    ACT Squares + ACT Identity linear factor; DVE does den + quadratic join;
    GPSIMD does both remaining products. Balances DVE/ACT/GPSIMD."""
    import concourse.mybir as mybir
    from concourse.tile import TileContext

    W = D // G
    e, (h1, k1), (h2, k2) = fac
    a5 = float(a[5]); lin_b = -a5 * e
    imm = lambda v: mybir.ImmediateValue(dtype=mybir.dt.float32, value=v)
    AF = mybir.ActivationFunctionType

    with TileContext(nc) as tc:
        with tc.tile_pool(name="const", bufs=1) as cpool, \
             tc.tile_pool(name="xo", bufs=4) as xpool, \
             tc.tile_pool(name="work", bufs=3) as pool:
            ct = cpool.tile([P, G + 4], f32)
            nc.sync.dma_start(out=ct[:], in_=coef[:, :])
            for i in range(N_TILES):
                r0 = i * P
                xt = xpool.tile([P, D], f32, tag="x")
                nc.sync.dma_start(out=xt[:], in_=x[r0:r0 + P, :])

                dent = pool.tile([P, D], f32, tag="den")
                for g in range(G):
                    sl = slice(g * W, (g + 1) * W)
                    nc.vector._custom_dve(
                        ops["KAT_DEN"],
                        out=dent[:, sl], in0=xt[:, sl], in1=ct[:, g:g + 1],
                        s0=float(c[g, 4]), s1=float(c[g, 3]), imm2=float(c[g, 2]),
                    )
                rt = pool.tile([P, D], f32, tag="r")
                nc.scalar.add_instruction(
                    mybir.InstActivation(
                        name=nc.get_next_instruction_name(),
                        func=AF.Reciprocal,
                        ins=[nc.scalar.lower_ap(dent[:]),
                             imm(0.0), imm(1.0), imm(0.0)],
                        outs=[nc.scalar.lower_ap(rt[:])],
                    )
                )
                if i < n_b:
                    at = pool.tile([P, D], f32, tag="qa")
                    nc.scalar.activation(at[:], xt[:], AF.Square,
                                         bias=ct[:, G + 1:G + 2])
                    bt = pool.tile([P, D], f32, tag="qb")
                    nc.scalar.activation(bt[:], xt[:], AF.Square,
                                         bias=ct[:, G + 2:G + 3])
                    lt = pool.tile([P, D], f32, tag="lin")
                    nc.scalar.activation(lt[:], xt[:], AF.Identity,
                                         bias=ct[:, G + 3:G + 4], scale=a5)
                    m1 = pool.tile([P, D], f32, tag="m")
                    nc.vector._custom_dve(
                        ops["KAT_FAC1"],
                        out=m1[:], in0=at[:], in1=bt[:],
                        s0=float(k1), s1=float(k2),
                    )
                    nc.gpsimd.tensor_mul(at[:], m1[:], lt[:])   # M1*lin
                    nc.gpsimd.tensor_mul(xt[:], at[:], rt[:])   # * recip
                else:
                    qt = pool.tile([P, D], f32, tag="qa")
                    nc.vector._custom_dve(
                        ops["KAT_NUMQ"],
                        out=qt[:], in0=xt[:], in1=ct[:, G:G + 1],
                        s0=float(a[5]), s1=float(a[4]), imm2=float(a[3]),
                    )
                    mt = pool.tile([P, D], f32, tag="m")
                    nc.vector._custom_dve(
                        ops["KAT_NUMM"],
                        out=mt[:], in0=qt[:], in1=xt[:],
                        s0=float(a[1]), s1=float(a[0]),
                    )
                    nc.gpsimd.tensor_mul(xt[:], mt[:], rt[:])
                nc.sync.dma_start(out=y[r0:r0 + P, :], in_=xt[:])
    nc.compile()
    return nc


def kernel(x, weight_numerator, weight_denominator, num_groups):
    from concourse import bass_utils

    x = np.ascontiguousarray(np.asarray(x, dtype=np.float32))
    a = np.asarray(weight_numerator, np.float32).reshape(-1)          # (6,)
    wd = np.asarray(weight_denominator, np.float32)                   # (G,4)
    G = int(num_groups)
    c = np.abs(np.concatenate([np.ones((G, 1), np.float32), wd], axis=1))

    nc = _build_module(a, c, G)

    coef_arr = np.zeros((P, G + 4), np.float32)
    coef_arr[:, :G] = c[:, 1][None, :]     # per-group c1 (spilled C3 of KAT_DEN)
    coef_arr[:, G] = a[2]                  # a2 (spilled C3 of KAT_NUMQ)
    fac = _factor_numerator(a)
    if fac is not None:                    # ACT biases for the mix variant
        _e, (_h1, _k1), (_h2, _k2) = fac
        coef_arr[:, G + 1] = _h1
        coef_arr[:, G + 2] = _h2
        coef_arr[:, G + 3] = -float(a[5]) * _e

    xr = x.reshape(B, N_CORES, L_SH, D)
    in_maps = [
        {"x": np.ascontiguousarray(xr[:, core]).reshape(ROWS, D),
         "coef": coef_arr}
        for core in range(N_CORES)
    ]
    res = bass_utils.run_bass_kernel_spmd(nc, in_maps, core_ids=list(range(N_CORES)))

    out = np.empty((B, N_CORES, L_SH, D), np.float32)
    for core in range(N_CORES):
        out[:, core] = res.results[core]["y"].reshape(B, L_SH, D)
    return out.reshape(B, L, D)



# revision 2
# speedup vs baseline: 1.5613x; 1.5613x over previous
"""KAT rational-group activation kernel for Trainium2 (Bass/Tile), 8-core SPMD.

Computes out = num(x) / den_g(x) elementwise over x:(4,4096,2048) f32, where
  num(x) = quintic (coeffs shared), den_g(x) = 1 + c1 x + ... + c4 x^4 per
  group g = channel // 256 (8 groups).

Fast path ("pf", partial-fraction): rewrite via polynomial division
  num/den = alpha*x + beta + R(x)/den(x),   deg R <= 3
then normalize with three free knobs so the device program needs only two
full custom-DVE passes per element:
  - lam_g  (host-side per-channel scale of x:  z = lam*x)
  - sig_r_g (folded into the ACT reciprocal's input scale)
  - sig2_g (host-side per-channel scale of the output)
chosen so that rho3~ = 1, rho0~ = 1, alpha~ = +/-1. Device per tile:
  1. custom DVE KAT_DEN:   den = (((c4''z+c3'')z+c2'')z+c1'')z + 1
  2. ACT Reciprocal:       r = 1/(sig_r * den)
  3. custom DVE KAT_PF_OUT: out = (((z+p2)z+p1)z+1)*r (+/-) z + b2
Host: out = sig2_g * out_dev.

Data layout: channels on partitions (host transposes x), sequence sharded
across 8 cores. I/O in fp16 (tolerance is 2e-2 relative to global max; fp16
end-to-end error measured ~7e-4). All SBUF intermediates f32.

Fallback path "opt" (previous kernel) is kept for degenerate coefficient
sets where the normalization is ill-conditioned.
"""

import numpy as np

B, L, D = 4, 4096, 2048
G_FIXED = 8
N_CORES = 8
P = 128                        # SBUF partitions
ROWS_ALL = B * L               # 16384 rows total
RW = ROWS_ALL // N_CORES       # 2048 rows per core (free dim)
NBLK = D // P                  # 16 channel blocks (partition tiles)

# legacy constants for the "opt" fallback (row-sharded layout)
L_SH = L // N_CORES
ROWS = B * L_SH
N_TILES = ROWS // P

_OPS_CACHE = {}


def _register_ops():
    """Define + register the KAT custom DVE ops (idempotent)."""
    if _OPS_CACHE:
        return _OPS_CACHE

    from concourse import dve_ops
    from concourse.dve_ops import DveOp
    from concourse.dve_spec import (
        C0, C1, C2, C3, One, Spec, Src0, Src1,
        _has_src1, _spill_c3_to_src1, lower,
    )
    from concourse.dve_uop import DveOpSpec

    # den = (((c4*z + c3)*z + c2)*z + c1)*z + 1   [C0..C2 imm, C3 -> in1 spill]
    den_body = _spill_c3_to_src1(
        (((C0 * Src0 + C1) * Src0 + C2) * Src0 + C3) * Src0 + One
    )
    den_ref = lambda in0, in1, s0, s1, imm2: (
        (((s0 * in0.astype(np.float32) + s1) * in0 + imm2) * in0
         + np.asarray(in1, np.float32).reshape(-1, 1)) * in0 + 1.0
    )

    # Q = ((a5*x + a4)*x + a3)*x + a2             [C0..C2 imm, C3 -> in1 spill]
    numq_body = _spill_c3_to_src1(
        ((C0 * Src0 + C1) * Src0 + C2) * Src0 + C3
    )
    numq_ref = lambda in0, in1, s0, s1, imm2: (
        ((s0 * in0.astype(np.float32) + s1) * in0 + imm2) * in0
        + np.asarray(in1, np.float32).reshape(-1, 1)
    )

    # M = (Q*x + a1)*x + a0                        [two full streams]
    numm_body = (Src0 * Src1 + C0) * Src1 + C1
    numm_ref = lambda in0, in1, s0, s1, imm2: (
        (in0.astype(np.float32) * in1 + s0) * in1 + s1
    )

    # out = (((z + p2)*z + p1)*z + 1)*r + z + b2   [partial-fraction tail, +z]
    pfp_body = ((((Src0 + C0) * Src0 + C1) * Src0 + One) * Src1) + Src0 + C2
    pfp_ref = lambda in0, in1, s0, s1, imm2: (
        (((in0.astype(np.float32) + s0) * in0 + s1) * in0 + 1.0) * in1
        + in0 + imm2
    )

    # out = (((z + p2)*z + p1)*z + 1)*r - z + b2   [partial-fraction tail, -z]
    pfn_body = ((((Src0 + C0) * Src0 + C1) * Src0 + One) * Src1) - Src0 + C2
    pfn_ref = lambda in0, in1, s0, s1, imm2: (
        (((in0.astype(np.float32) + s0) * in0 + s1) * in0 + 1.0) * in1
        - in0 + imm2
    )

    defs = [
        ("KAT_DEN", den_body, den_ref),
        ("KAT_NUMQ", numq_body, numq_ref),
        ("KAT_NUMM", numm_body, numm_ref),
        ("KAT_PF_OUTP", pfp_body, pfp_ref),
        ("KAT_PF_OUTN", pfn_body, pfn_ref),
    ]

    existing = {op.name for op in dve_ops.OPS}
    for name, body, ref in defs:
        if name in existing:
            _OPS_CACHE[name] = next(op for op in dve_ops.OPS if op.name == name)
            continue
        spec = Spec(body=body, reference=ref)
        row = max(dve_ops._SUB_OPCODE_FOR_NAME.values()) + 1
        assert row < 0x20, "custom DVE row field overflow"
        dve_ops._SUB_OPCODE_FOR_NAME[name] = row
        shas = {}
        for ver in ("v3", "v4"):
            uops = lower(spec, ver=ver)
            shas[ver] = DveOpSpec(
                name=name, opcode=row, uops=uops, rd1_en=_has_src1(spec)
            ).sha(ver)
        op = DveOp(name, spec, subdim=False, uops_sha=shas)
        dve_ops.OPS.append(op)
        dve_ops.CUSTOM_DVE_SPECS[name] = spec
        _OPS_CACHE[name] = op
    return _OPS_CACHE


def derive_pf_params(a, c):
    """Per-group partial-fraction constants, or None if ill-conditioned.

    a: (6,) numerator coeffs a0..a5. c: (G,5) denominator coeffs c0..c4
    (c0 == 1). Returns list of dicts per group with keys:
      lam, sig2, sig_r, rho2t, rho1t, beta2, cden (c1''..c4''), pos (bool).
    """
    a = np.asarray(a, np.float64).reshape(-1)
    c = np.asarray(c, np.float64)
    G = c.shape[0]
    out = []
    for g in range(G):
        cg = c[g]
        if abs(cg[4]) < 1e-12:
            return None
        q, r = np.polydiv(a[::-1], cg[::-1])
        if len(q) != 2:
            return None
        alpha, beta = q[0], q[1]
        R = r[::-1]
        R = np.pad(R, (0, 4 - len(R)))
        rho0, rho1, rho2, rho3 = R
        if abs(rho0) < 1e-10 or abs(rho3) < 1e-12 or abs(alpha) < 1e-10:
            return None
        lam = np.cbrt(rho3 / rho0)
        if not (2.0**-6 < abs(lam) < 2.0**6):
            return None
        sig2 = alpha / lam          # alpha~ = +1 variant
        sig_r = sig2 / rho0
        pos = True
        if sig_r < 0:               # flip to alpha~ = -1 so sig_r > 0
            sig2, sig_r, pos = -sig2, -sig_r, False
        if not (2.0**-9 < abs(sig2) < 2.0**14):
            return None
        rho2t = rho2 / (rho0 * lam * lam)
        rho1t = rho1 / (rho0 * lam)
        beta2 = beta / sig2
        # den coeffs in z = lam*x coordinates: c_k'' = c_k / lam^k
        cden = cg[1:5] / lam ** np.arange(1, 5)
        vals = [lam, sig2, sig_r, rho2t, rho1t, beta2, *cden]
        if not all(np.isfinite(vals)):
            return None
        out.append(dict(lam=lam, sig2=sig2, sig_r=sig_r, rho2t=rho2t,
                        rho1t=rho1t, beta2=beta2, cden=cden, pos=pos))
    return out


VARIANT = "pf"  # "pf" fast path; "opt" fallback


def _build_module(a, c, G, variant=None):
    """Trace the per-core Bass module. a:(6,) numerator, c:(G,5) |den| coeffs."""
    variant = VARIANT if variant is None else variant
    if variant == "pf":
        params = derive_pf_params(a, c)
        if params is not None and G == G_FIXED:
            return _build_pf(params)
        variant = "opt"
    return _build_opt_module(a, c, G)


def _build_pf(params):
    """Partial-fraction module: [2048 ch, 2048 rows] fp16 in/out per core.

    Channels on partitions; each of the 16 partition tiles lies in a single
    group, so all per-group constants are instruction immediates.
    """
    import concourse.bacc as bacc
    import concourse.mybir as mybir
    from concourse.tile import TileContext

    ops = _register_ops()
    f32 = mybir.dt.float32
    f16 = mybir.dt.float16
    imm = lambda v: mybir.ImmediateValue(dtype=mybir.dt.float32, value=float(v))

    nc = bacc.Bacc("TRN2", target_bir_lowering=False)
    x = nc.dram_tensor("x", (D, RW), f16, kind="ExternalInput")
    coef = nc.dram_tensor("coef", (P, NBLK), f32, kind="ExternalInput")
    y = nc.dram_tensor("y", (D, RW), f16, kind="ExternalOutput")

    with TileContext(nc) as tc:
        with tc.tile_pool(name="const", bufs=1) as cpool, \
             tc.tile_pool(name="x", bufs=3) as xpool, \
             tc.tile_pool(name="den", bufs=2) as dpool, \
             tc.tile_pool(name="rec", bufs=2) as rpool, \
             tc.tile_pool(name="out", bufs=2) as opool:
            ct = cpool.tile([P, NBLK], f32)
            nc.sync.dma_start(out=ct[:], in_=coef[:, :])
            for b in range(NBLK):
                g = b * P // (D // G_FIXED)
                pg = params[g]
                xt = xpool.tile([P, RW], f16, tag="x")
                nc.sync.dma_start(out=xt[:], in_=x[b * P:(b + 1) * P, :])

                dent = dpool.tile([P, RW], f32, tag="d")
                nc.vector._custom_dve(
                    ops["KAT_DEN"],
                    out=dent[:], in0=xt[:], in1=ct[:, b:b + 1],
                    s0=float(pg["cden"][3]), s1=float(pg["cden"][2]),
                    imm2=float(pg["cden"][1]),
                )
                rt = rpool.tile([P, RW], f32, tag="r")
                nc.scalar.add_instruction(
                    mybir.InstActivation(
                        name=nc.get_next_instruction_name(),
                        func=mybir.ActivationFunctionType.Reciprocal,
                        ins=[nc.scalar.lower_ap(dent[:]),
                             imm(0.0), imm(pg["sig_r"]), imm(0.0)],
                        outs=[nc.scalar.lower_ap(rt[:])],
                    )
                )
                ot = opool.tile([P, RW], f16, tag="o")
                nc.vector._custom_dve(
                    ops["KAT_PF_OUTP" if pg["pos"] else "KAT_PF_OUTN"],
                    out=ot[:], in0=xt[:], in1=rt[:],
                    s0=float(pg["rho2t"]), s1=float(pg["rho1t"]),
                    imm2=float(pg["beta2"]),
                )
                nc.sync.dma_start(out=y[b * P:(b + 1) * P, :], in_=ot[:])
    nc.compile()
    return nc


def _kernel_pf(x, a, c):
    """Fast path driver. x:(B,L,D) f32. Returns (B,L,D) f32 or None."""
    from concourse import bass_utils

    params = derive_pf_params(a, c)
    if params is None:
        return None
    nc = _build_pf(params)

    Wg = D // G_FIXED
    lam_ch = np.repeat([p["lam"] for p in params], Wg)      # (D,)
    sig2_ch = np.repeat([p["sig2"] for p in params], Wg)    # (D,)

    # host: z = lam * x, transposed to [D, B*L], fp16
    x2 = np.asarray(x, np.float32).reshape(ROWS_ALL, D)
    zt = (x2.T * lam_ch[:, None].astype(np.float32)).astype(np.float16)

    coef_arr = np.zeros((P, NBLK), np.float32)
    for b in range(NBLK):
        g = b * P // Wg
        coef_arr[:, b] = params[g]["cden"][0]   # c1'' spilled via in1

    in_maps = [
        {"x": np.ascontiguousarray(zt[:, core * RW:(core + 1) * RW]),
         "coef": coef_arr}
        for core in range(N_CORES)
    ]
    res = bass_utils.run_bass_kernel_spmd(nc, in_maps, core_ids=list(range(N_CORES)))

    out2 = np.empty((ROWS_ALL, D), np.float32)
    s32 = sig2_ch.astype(np.float32)[:, None]
    for core in range(N_CORES):
        ys = res.results[core]["y"]             # [D, RW] f16
        out2[core * RW:(core + 1) * RW, :] = (ys.astype(np.float32) * s32).T
    return out2.reshape(B, L, D)


# ---------------------------------------------------------------------------
# Fallback "opt" path (previous kernel): row-sharded f32, den/recip/num/mul.
# ---------------------------------------------------------------------------

def _build_opt_module(a, c, G):
    import concourse.bacc as bacc
    import concourse.mybir as mybir
    from concourse.tile import TileContext

    ops = _register_ops()
    f32 = mybir.dt.float32
    W = D // G
    imm = lambda v: mybir.ImmediateValue(dtype=mybir.dt.float32, value=v)

    nc = bacc.Bacc("TRN2", target_bir_lowering=False)
    x = nc.dram_tensor("x", (ROWS, D), f32, kind="ExternalInput")
    coef = nc.dram_tensor("coef", (P, G + 4), f32, kind="ExternalInput")
    y = nc.dram_tensor("y", (ROWS, D), f32, kind="ExternalOutput")

    with TileContext(nc) as tc:
        with tc.tile_pool(name="const", bufs=1) as cpool, \
             tc.tile_pool(name="xo", bufs=4) as xpool, \
             tc.tile_pool(name="work", bufs=3) as pool:
            ct = cpool.tile([P, G + 4], f32)
            nc.sync.dma_start(out=ct[:], in_=coef[:, :])
            for i in range(N_TILES):
                r0 = i * P
                xt = xpool.tile([P, D], f32, tag="x")
                nc.sync.dma_start(out=xt[:], in_=x[r0:r0 + P, :])
                dent = pool.tile([P, D], f32, tag="den")
                for g in range(G):
                    sl = slice(g * W, (g + 1) * W)
                    nc.vector._custom_dve(
                        ops["KAT_DEN"],
                        out=dent[:, sl], in0=xt[:, sl], in1=ct[:, g:g + 1],
                        s0=float(c[g, 4]), s1=float(c[g, 3]), imm2=float(c[g, 2]),
                    )
                rt = pool.tile([P, D], f32, tag="r")
                nc.scalar.add_instruction(
                    mybir.InstActivation(
                        name=nc.get_next_instruction_name(),
                        func=mybir.ActivationFunctionType.Reciprocal,
                        ins=[nc.scalar.lower_ap(dent[:]),
                             imm(0.0), imm(1.0), imm(0.0)],
                        outs=[nc.scalar.lower_ap(rt[:])],
                    )
                )
                qt = pool.tile([P, D], f32, tag="q")
                nc.vector._custom_dve(
                    ops["KAT_NUMQ"],
                    out=qt[:], in0=xt[:], in1=ct[:, G:G + 1],
                    s0=float(a[5]), s1=float(a[4]), imm2=float(a[3]),
                )
                mt = pool.tile([P, D], f32, tag="m")
                nc.vector._custom_dve(
                    ops["KAT_NUMM"],
                    out=mt[:], in0=qt[:], in1=xt[:],
                    s0=float(a[1]), s1=float(a[0]),
                )
                ot = xt
                nc.gpsimd.tensor_mul(ot[:], mt[:], rt[:])
                nc.sync.dma_start(out=y[r0:r0 + P, :], in_=ot[:])
    nc.compile()
    return nc


def _kernel_opt(x, a, c, G):
    from concourse import bass_utils

    nc = _build_opt_module(a, c, G)
    coef_arr = np.zeros((P, G + 4), np.float32)
    coef_arr[:, :G] = c[:, 1][None, :]
    coef_arr[:, G] = a[2]

    xr = np.asarray(x, np.float32).reshape(B, N_CORES, L_SH, D)
    in_maps = [
        {"x": np.ascontiguousarray(xr[:, core]).reshape(ROWS, D),
         "coef": coef_arr}
        for core in range(N_CORES)
    ]
    res = bass_utils.run_bass_kernel_spmd(nc, in_maps, core_ids=list(range(N_CORES)))
    out = np.empty((B, N_CORES, L_SH, D), np.float32)
    for core in range(N_CORES):
        out[:, core] = res.results[core]["y"].reshape(B, L_SH, D)
    return out.reshape(B, L, D)


def kernel(x, weight_numerator, weight_denominator, num_groups):
    x = np.ascontiguousarray(np.asarray(x, dtype=np.float32))
    a = np.asarray(weight_numerator, np.float32).reshape(-1)          # (6,)
    wd = np.asarray(weight_denominator, np.float32)                   # (G,4)
    G = int(num_groups)
    c = np.abs(np.concatenate([np.ones((G, 1), np.float32), wd], axis=1))

    if VARIANT == "pf" and G == G_FIXED and x.shape == (B, L, D):
        out = _kernel_pf(x, a, c)
        if out is not None:
            return out
    return _kernel_opt(x, a, c, G)


# revision 19
# speedup vs baseline: 1.8700x; 1.1978x over previous
"""KAT rational-group activation kernel for Trainium2 (Bass/Tile), 8-core SPMD.

Computes out = num(x) / den_g(x) elementwise over x:(4,4096,2048) f32, where
  num(x) = quintic (coeffs shared), den_g(x) = 1 + c1 x + ... + c4 x^4 per
  group g = channel // 256 (8 groups).

Fast path ("pf", partial-fraction): rewrite via polynomial division
  num/den = alpha*x + beta + R(x)/den(x),   deg R <= 3
then normalize with three free knobs so the device program needs only two
full custom-DVE passes per element:
  - lam_g  (host-side per-channel scale of x:  z = lam*x)
  - sig_r_g (folded into the ACT reciprocal's input scale)
  - sig2_g (host-side per-channel scale of the output)
chosen so that rho3~ = 1, rho0~ = 1, alpha~ = +/-1. Device per tile:
  1. custom DVE KAT_DEN:   den = (((c4''z+c3'')z+c2'')z+c1'')z + 1
  2. ACT Reciprocal:       r = 1/(sig_r * den)
  3. custom DVE KAT_PF_OUT: out = (((z+p2)z+p1)z+1)*r (+/-) z + b2
Host: out = sig2_g * out_dev.

Data layout: channels on partitions (host transposes x), sequence sharded
across 8 cores. I/O in fp16 (tolerance is 2e-2 relative to global max; fp16
end-to-end error measured ~7e-4). All SBUF intermediates f32.

Fallback path "opt" (previous kernel) is kept for degenerate coefficient
sets where the normalization is ill-conditioned.
"""

import numpy as np

B, L, D = 4, 4096, 2048
G_FIXED = 8
N_CORES = 8
P = 128                        # SBUF partitions
ROWS_ALL = B * L               # 16384 rows total
RW = ROWS_ALL // N_CORES       # 2048 rows per core (free dim)
NBLK = D // P                  # 16 channel blocks (partition tiles)

# legacy constants for the "opt" fallback (row-sharded layout)
L_SH = L // N_CORES
ROWS = B * L_SH
N_TILES = ROWS // P

_OPS_CACHE = {}


def _register_ops():
    """Define + register the KAT custom DVE ops (idempotent)."""
    if _OPS_CACHE:
        return _OPS_CACHE

    from concourse import dve_ops
    from concourse.dve_ops import DveOp
    from concourse.dve_spec import (
        C0, C1, C2, C3, One, Spec, Src0, Src1,
        _has_src1, _spill_c3_to_src1, lower,
    )
    from concourse.dve_uop import DveOpSpec

    # den = (((c4*z + c3)*z + c2)*z + c1)*z + 1   [C0..C2 imm, C3 -> in1 spill]
    den_body = _spill_c3_to_src1(
        (((C0 * Src0 + C1) * Src0 + C2) * Src0 + C3) * Src0 + One
    )
    den_ref = lambda in0, in1, s0, s1, imm2: (
        (((s0 * in0.astype(np.float32) + s1) * in0 + imm2) * in0
         + np.asarray(in1, np.float32).reshape(-1, 1)) * in0 + 1.0
    )

    # Q = ((a5*x + a4)*x + a3)*x + a2             [C0..C2 imm, C3 -> in1 spill]
    numq_body = _spill_c3_to_src1(
        ((C0 * Src0 + C1) * Src0 + C2) * Src0 + C3
    )
    numq_ref = lambda in0, in1, s0, s1, imm2: (
        ((s0 * in0.astype(np.float32) + s1) * in0 + imm2) * in0
        + np.asarray(in1, np.float32).reshape(-1, 1)
    )

    # M = (Q*x + a1)*x + a0                        [two full streams]
    numm_body = (Src0 * Src1 + C0) * Src1 + C1
    numm_ref = lambda in0, in1, s0, s1, imm2: (
        (in0.astype(np.float32) * in1 + s0) * in1 + s1
    )

    # out = (((z + p2)*z + p1)*z + 1)*r + z + b2   [partial-fraction tail, +z]
    pfp_body = ((((Src0 + C0) * Src0 + C1) * Src0 + One) * Src1) + Src0 + C2
    pfp_ref = lambda in0, in1, s0, s1, imm2: (
        (((in0.astype(np.float32) + s0) * in0 + s1) * in0 + 1.0) * in1
        + in0 + imm2
    )

    # out = (((z + p2)*z + p1)*z + 1)*r - z + b2   [partial-fraction tail, -z]
    pfn_body = ((((Src0 + C0) * Src0 + C1) * Src0 + One) * Src1) - Src0 + C2
    pfn_ref = lambda in0, in1, s0, s1, imm2: (
        (((in0.astype(np.float32) + s0) * in0 + s1) * in0 + 1.0) * in1
        - in0 + imm2
    )

    defs = [
        ("KAT_DEN", den_body, den_ref),
        ("KAT_NUMQ", numq_body, numq_ref),
        ("KAT_NUMM", numm_body, numm_ref),
        ("KAT_PF_OUTP", pfp_body, pfp_ref),
        ("KAT_PF_OUTN", pfn_body, pfn_ref),
    ]

    existing = {op.name for op in dve_ops.OPS}
    for name, body, ref in defs:
        if name in existing:
            _OPS_CACHE[name] = next(op for op in dve_ops.OPS if op.name == name)
            continue
        spec = Spec(body=body, reference=ref)
        row = max(dve_ops._SUB_OPCODE_FOR_NAME.values()) + 1
        assert row < 0x20, "custom DVE row field overflow"
        dve_ops._SUB_OPCODE_FOR_NAME[name] = row
        shas = {}
        for ver in ("v3", "v4"):
            uops = lower(spec, ver=ver)
            shas[ver] = DveOpSpec(
                name=name, opcode=row, uops=uops, rd1_en=_has_src1(spec)
            ).sha(ver)
        op = DveOp(name, spec, subdim=False, uops_sha=shas)
        dve_ops.OPS.append(op)
        dve_ops.CUSTOM_DVE_SPECS[name] = spec
        _OPS_CACHE[name] = op
    return _OPS_CACHE


def derive_pf_params(a, c):
    """Per-group partial-fraction constants, or None if ill-conditioned.

    a: (6,) numerator coeffs a0..a5. c: (G,5) denominator coeffs c0..c4
    (c0 == 1). Returns list of dicts per group with keys:
      lam, sig2, sig_r, rho2t, rho1t, beta2, cden (c1''..c4''), pos (bool).
    """
    a = np.asarray(a, np.float64).reshape(-1)
    c = np.asarray(c, np.float64)
    G = c.shape[0]
    out = []
    for g in range(G):
        cg = c[g]
        if abs(cg[4]) < 1e-12:
            return None
        q, r = np.polydiv(a[::-1], cg[::-1])
        if len(q) != 2:
            return None
        alpha, beta = q[0], q[1]
        R = r[::-1]
        R = np.pad(R, (0, 4 - len(R)))
        rho0, rho1, rho2, rho3 = R
        if abs(rho0) < 1e-10 or abs(rho3) < 1e-12 or abs(alpha) < 1e-10:
            return None
        lam = np.cbrt(rho3 / rho0)
        if not (2.0**-6 < abs(lam) < 2.0**6):
            return None
        sig2 = alpha / lam          # alpha~ = +1 variant
        sig_r = sig2 / rho0
        pos = True
        if sig_r < 0:               # flip to alpha~ = -1 so sig_r > 0
            sig2, sig_r, pos = -sig2, -sig_r, False
        if not (2.0**-9 < abs(sig2) < 2.0**14):
            return None
        rho2t = rho2 / (rho0 * lam * lam)
        rho1t = rho1 / (rho0 * lam)
        beta2 = beta / sig2
        # den coeffs in z = lam*x coordinates: c_k'' = c_k / lam^k
        cden = cg[1:5] / lam ** np.arange(1, 5)
        vals = [lam, sig2, sig_r, rho2t, rho1t, beta2, *cden]
        if not all(np.isfinite(vals)):
            return None
        d = dict(lam=lam, sig2=sig2, sig_r=sig_r, rho2t=rho2t,
                 rho1t=rho1t, beta2=beta2, cden=cden, pos=pos, fac=None)
        # factored den for the ACT/Pool offload path:
        #   den'' = c4''*((z+h1)^2+k1)*((z+h2)^2+k2)
        d["fac"] = _factor_quartic(cden, lam)
        out.append(d)
    return out


def _factor_quartic(cden, lam):
    """Factor 1 + c1''z + ... + c4''z^4 into c4''*(z^2+p1z+q1)(z^2+p2z+q2).
    Returns (h1,k1,h2,k2) with quadratic = (z+h)^2 + k, or None."""
    try:
        roots = np.roots([cden[3], cden[2], cden[1], cden[0], 1.0])
    except Exception:
        return None
    if len(roots) != 4:
        return None
    cplx = [r for r in roots if abs(r.imag) > 1e-9]
    reals = sorted(r.real for r in roots if abs(r.imag) <= 1e-9)
    quads = []
    used = set()
    for i, z1 in enumerate(cplx):
        if i in used:
            continue
        for j in range(i + 1, len(cplx)):
            if j not in used and abs(np.conj(z1) - cplx[j]) < 1e-6 * max(1, abs(z1)):
                quads.append((-2 * z1.real, abs(z1) ** 2))
                used.add(i)
                used.add(j)
                break
    while len(reals) >= 2:
        r1 = reals.pop(0)
        r2 = reals.pop(-1)
        quads.append((-(r1 + r2), r1 * r2))
    if len(quads) != 2:
        return None
    (p1, q1), (p2, q2) = quads
    h1, k1 = p1 / 2, q1 - p1 * p1 / 4
    h2, k2 = p2 / 2, q2 - p2 * p2 / 4
    if not all(np.isfinite([h1, k1, h2, k2])):
        return None
    # validate on the data range
    zz = np.linspace(-5.8 * abs(lam), 5.8 * abs(lam), 4001)
    den_h = (((cden[3] * zz + cden[2]) * zz + cden[1]) * zz + cden[0]) * zz + 1.0
    den_f = cden[3] * ((zz + h1) ** 2 + k1) * ((zz + h2) ** 2 + k2)
    if np.abs(den_f - den_h).max() > 1e-5 * np.abs(den_h).min():
        return None
    return (float(h1), float(k1), float(h2), float(k2))


VARIANT = "pf"  # "pf" fast path; "opt" fallback
# full blocks whose den is computed on ACT/Pool instead of DVE
OFFLOAD_BLOCKS = (2, 5, 8, 11)
OFF_DELAY = 3
RECIP_PRIO = 0
RR_ON_DVE = False
STT_DEN = False


def _build_module(a, c, G, variant=None):
    """Trace the per-core Bass module. a:(6,) numerator, c:(G,5) |den| coeffs."""
    variant = VARIANT if variant is None else variant
    if variant == "pf":
        params = derive_pf_params(a, c)
        if params is not None and G == G_FIXED:
            return _build_pf(params)
        variant = "opt"
    return _build_opt_module(a, c, G)


def _build_pf(params):
    """Partial-fraction module: [2048 ch, 2048 rows] fp16 in/out per core.

    Channels on partitions; each of the 16 partition tiles lies in a single
    group, so all per-group constants are instruction immediates.
    """
    import concourse.bacc as bacc
    import concourse.mybir as mybir
    from concourse.tile import TileContext

    ops = _register_ops()
    f32 = mybir.dt.float32
    f16 = mybir.dt.float16
    imm = lambda v: mybir.ImmediateValue(dtype=mybir.dt.float32, value=float(v))

    nc = bacc.Bacc("TRN2", target_bir_lowering=False)
    x = nc.dram_tensor("x", (D, RW), f16, kind="ExternalInput")
    coef = nc.dram_tensor("coef", (P, 3 * NBLK), f32, kind="ExternalInput")
    y = nc.dram_tensor("y", (D, RW), f16, kind="ExternalOutput")

    # graduated pieces: small at the head (fast pipeline fill) and at the
    # tail (short serial drain chain); full-size tiles mid-stream.
    pieces = []                    # (channel block, row start, row count)
    for b in range(NBLK):
        if b == 0:
            splits = [RW // 4] * 4
        elif b == NBLK - 1:
            splits = [RW // 2] + [RW // 4] * 2
        else:
            splits = [RW]
        r0 = 0
        for n in splits:
            pieces.append((b, r0, n))
            r0 += n

    def group_of(b):
        return b * P // (D // G_FIXED)

    # den offload (ACT Squares + Pool STT) for these full blocks, when the
    # group's quartic factorization is available
    def fac_ok(g):
        f = params[g]["fac"]
        return f is not None and f[1] > 1e-3 and f[3] > 1e-3
    offload = {b for b in OFFLOAD_BLOCKS if 0 < b < NBLK - 1 and fac_ok(group_of(b))}
    AF = mybir.ActivationFunctionType
    ALU = mybir.AluOpType

    full = RW

    with TileContext(nc) as tc:
        with tc.tile_pool(name="const", bufs=1) as cpool, \
             tc.tile_pool(name="x", bufs=1) as xpool, \
             tc.tile_pool(name="den", bufs=1) as dpool, \
             tc.tile_pool(name="rec", bufs=1) as rpool, \
             tc.tile_pool(name="out", bufs=1) as opool, \
             tc.tile_pool(name="sqa", bufs=1) as apool, \
             tc.tile_pool(name="sqb", bufs=1) as bpool, \
             tc.tile_pool(name="w1", bufs=1) as wpool:
            ct = cpool.tile([P, 3 * NBLK], f32)
            nc.scalar.dma_start(out=ct[:], in_=coef[:, :])

            off_state = {}   # b -> (xt, w1_tile)

            def emit_off_den(b, n, r0):
                """den offload: ACT Squares + ACT per-factor reciprocals
                (k folded into recip bias), Pool multiplies the factors.
                Leaves rr = 1/(sig_r*den) ready for the c2 tail."""
                g = group_of(b)
                pg = params[g]
                xt = xpool.tile([P, n], f16, tag="xo", bufs=5)
                nc.sync.dma_start(out=xt[:], in_=x[b * P:(b + 1) * P, r0:r0 + n])
                h1, k1, h2, k2 = pg["fac"]
                sc = pg["sig_r"] * pg["cden"][3]
                at = apool.tile([P, n], f16, tag="a", bufs=2)
                nc.scalar.activation(at[:], xt[:], AF.Square,
                                     bias=ct[:, NBLK + 2 * b:NBLK + 2 * b + 1])
                bt = bpool.tile([P, n], f16, tag="b", bufs=2)
                nc.scalar.activation(bt[:], xt[:], AF.Square,
                                     bias=ct[:, NBLK + 2 * b + 1:NBLK + 2 * b + 2])
                r1 = apool.tile([P, n], f16, tag="r1", bufs=2)
                nc.scalar.add_instruction(
                    mybir.InstActivation(
                        name=nc.get_next_instruction_name(),
                        func=mybir.ActivationFunctionType.Reciprocal,
                        ins=[nc.scalar.lower_ap(at[:]),
                             imm(k1), imm(1.0), imm(0.0)],
                        outs=[nc.scalar.lower_ap(r1[:])],
                    )
                )
                r2 = bpool.tile([P, n], f16, tag="r2", bufs=2)
                nc.scalar.add_instruction(
                    mybir.InstActivation(
                        name=nc.get_next_instruction_name(),
                        func=mybir.ActivationFunctionType.Reciprocal,
                        ins=[nc.scalar.lower_ap(bt[:]),
                             imm(sc * k2), imm(sc), imm(0.0)],
                        outs=[nc.scalar.lower_ap(r2[:])],
                    )
                )
                rr = wpool.tile([P, n], f16, tag="rr", bufs=4)
                if RR_ON_DVE:
                    nc.vector.tensor_mul(rr[:], r1[:], r2[:])
                else:
                    nc.gpsimd.tensor_mul(rr[:], r1[:], r2[:])
                off_state[b] = (xt, rr)

            def emit_tail(b, n, r0, xt, dent, rscale, rt=None, rbias=0.0):
                g = group_of(b)
                pg = params[g]
                if rt is None:
                    rt = rpool.tile([P, n], f32, tag=f"r{n}",
                                    bufs=4 if n == full else 2)
                    nc.scalar.add_instruction(
                        mybir.InstActivation(
                            name=nc.get_next_instruction_name(),
                            func=mybir.ActivationFunctionType.Reciprocal,
                            ins=[nc.scalar.lower_ap(dent[:]),
                                 imm(rbias), imm(rscale), imm(0.0)],
                            outs=[nc.scalar.lower_ap(rt[:])],
                        )
                    )
                ot = opool.tile([P, n], f16, tag=f"o{n}",
                                bufs=3 if n == full else 2)
                nc.vector._custom_dve(
                    ops["KAT_PF_OUTP" if pg["pos"] else "KAT_PF_OUTN"],
                    out=ot[:], in0=xt[:], in1=rt[:],
                    s0=float(pg["rho2t"]), s1=float(pg["rho1t"]),
                    imm2=float(pg["beta2"]),
                )
                nc.sync.dma_start(out=y[b * P:(b + 1) * P, r0:r0 + n], in_=ot[:])

            def emit_main(b, n, r0):
                g = group_of(b)
                pg = params[g]
                c1pp, c2pp, c3pp, c4pp = [float(v) for v in pg["cden"]]
                xt = xpool.tile([P, n], f16, tag=f"x{n}",
                                bufs=5 if n == full else 4)
                nc.sync.dma_start(out=xt[:], in_=x[b * P:(b + 1) * P, r0:r0 + n])
                if STT_DEN:
                    # monic-quartic prefix via 3 fp16 STT ops (4x DVE mode);
                    # den = c4''*v3 + 1 folds into the reciprocal's scale+bias
                    v1 = dpool.tile([P, n], f16, tag=f"v1{n}", bufs=2)
                    nc.vector.scalar_tensor_tensor(
                        out=v1[:], in0=xt[:], scalar=c3pp / c4pp, in1=xt[:],
                        op0=ALU.add, op1=ALU.mult)
                    v2 = dpool.tile([P, n], f16, tag=f"v2{n}", bufs=2)
                    nc.vector.scalar_tensor_tensor(
                        out=v2[:], in0=v1[:], scalar=c2pp / c4pp, in1=xt[:],
                        op0=ALU.add, op1=ALU.mult)
                    dent = dpool.tile([P, n], f16, tag=f"v3{n}", bufs=3)
                    nc.vector.scalar_tensor_tensor(
                        out=dent[:], in0=v2[:], scalar=c1pp / c4pp, in1=xt[:],
                        op0=ALU.add, op1=ALU.mult)
                    sc = pg["sig_r"] * c4pp
                    emit_tail(b, n, r0, xt, dent, sc, rbias=pg["sig_r"])
                else:
                    dent = dpool.tile([P, n], f32, tag=f"d{n}",
                                      bufs=3 if n == full else 2)
                    nc.vector._custom_dve(
                        ops["KAT_DEN"],
                        out=dent[:], in0=xt[:], in1=ct[:, b:b + 1],
                        s0=c4pp, s1=c3pp, imm2=c2pp,
                    )
                    emit_tail(b, n, r0, xt, dent, pg["sig_r"])

            def pop_tail(ob):
                xt, rr = off_state.pop(ob)
                emit_tail(ob, RW, 0, xt, None, 0.0, rt=rr)

            pending = []          # (block, emit piece-index)
            for idx, (b, r0, n) in enumerate(pieces):
                # pop deferred tails once their Pool chain is ~OFF_DELAY
                # pieces old, so they never trail the graduated drain pieces
                while pending and idx - pending[0][1] >= OFF_DELAY:
                    pop_tail(pending.pop(0)[0])
                if b in offload:
                    emit_off_den(b, n, r0)
                    pending.append((b, idx))
                    continue
                emit_main(b, r0=r0, n=n)
            for ob, _ in pending:
                pop_tail(ob)
    nc.compile()
    return nc


def _kernel_pf(x, a, c):
    """Fast path driver. x:(B,L,D) f32. Returns (B,L,D) f32 or None."""
    from concourse import bass_utils

    params = derive_pf_params(a, c)
    if params is None:
        return None
    nc = _build_pf(params)

    Wg = D // G_FIXED
    lam_ch = np.repeat([p["lam"] for p in params], Wg)      # (D,)
    sig2_ch = np.repeat([p["sig2"] for p in params], Wg)    # (D,)

    # host: z = lam * x, transposed to [D, B*L], fp16
    x2 = np.asarray(x, np.float32).reshape(ROWS_ALL, D)
    zt = (x2.T * lam_ch[:, None].astype(np.float32)).astype(np.float16)

    coef_arr = np.zeros((P, 3 * NBLK), np.float32)
    for b in range(NBLK):
        g = b * P // Wg
        coef_arr[:, b] = params[g]["cden"][0]   # c1'' spilled via in1
        if params[g]["fac"] is not None:
            h1, k1, h2, k2 = params[g]["fac"]
            coef_arr[:, NBLK + 2 * b] = h1
            coef_arr[:, NBLK + 2 * b + 1] = h2

    in_maps = [
        {"x": np.ascontiguousarray(zt[:, core * RW:(core + 1) * RW]),
         "coef": coef_arr}
        for core in range(N_CORES)
    ]
    res = bass_utils.run_bass_kernel_spmd(nc, in_maps, core_ids=list(range(N_CORES)))

    out2 = np.empty((ROWS_ALL, D), np.float32)
    s32 = sig2_ch.astype(np.float32)[:, None]
    for core in range(N_CORES):
        ys = res.results[core]["y"]             # [D, RW] f16
        out2[core * RW:(core + 1) * RW, :] = (ys.astype(np.float32) * s32).T
    return out2.reshape(B, L, D)


# ---------------------------------------------------------------------------
# Fallback "opt" path (previous kernel): row-sharded f32, den/recip/num/mul.
# ---------------------------------------------------------------------------

def _build_opt_module(a, c, G):
    import concourse.bacc as bacc
    import concourse.mybir as mybir
    from concourse.tile import TileContext

    ops = _register_ops()
    f32 = mybir.dt.float32
    W = D // G
    imm = lambda v: mybir.ImmediateValue(dtype=mybir.dt.float32, value=v)

    nc = bacc.Bacc("TRN2", target_bir_lowering=False)
    x = nc.dram_tensor("x", (ROWS, D), f32, kind="ExternalInput")
    coef = nc.dram_tensor("coef", (P, G + 4), f32, kind="ExternalInput")
    y = nc.dram_tensor("y", (ROWS, D), f32, kind="ExternalOutput")

    with TileContext(nc) as tc:
        with tc.tile_pool(name="const", bufs=1) as cpool, \
             tc.tile_pool(name="xo", bufs=4) as xpool, \
             tc.tile_pool(name="work", bufs=3) as pool:
            ct = cpool.tile([P, G + 4], f32)
            nc.sync.dma_start(out=ct[:], in_=coef[:, :])
            for i in range(N_TILES):
                r0 = i * P
                xt = xpool.tile([P, D], f32, tag="x")
                nc.sync.dma_start(out=xt[:], in_=x[r0:r0 + P, :])
                dent = pool.tile([P, D], f32, tag="den")
                for g in range(G):
                    sl = slice(g * W, (g + 1) * W)
                    nc.vector._custom_dve(
                        ops["KAT_DEN"],
                        out=dent[:, sl], in0=xt[:, sl], in1=ct[:, g:g + 1],
                        s0=float(c[g, 4]), s1=float(c[g, 3]), imm2=float(c[g, 2]),
                    )
                rt = pool.tile([P, D], f32, tag="r")
                nc.scalar.add_instruction(
                    mybir.InstActivation(
                        name=nc.get_next_instruction_name(),
                        func=mybir.ActivationFunctionType.Reciprocal,
                        ins=[nc.scalar.lower_ap(dent[:]),
                             imm(0.0), imm(1.0), imm(0.0)],
                        outs=[nc.scalar.lower_ap(rt[:])],
                    )
                )
                qt = pool.tile([P, D], f32, tag="q")
                nc.vector._custom_dve(
                    ops["KAT_NUMQ"],
                    out=qt[:], in0=xt[:], in1=ct[:, G:G + 1],
                    s0=float(a[5]), s1=float(a[4]), imm2=float(a[3]),
                )
                mt = pool.tile([P, D], f32, tag="m")
                nc.vector._custom_dve(
                    ops["KAT_NUMM"],
                    out=mt[:], in0=qt[:], in1=xt[:],
                    s0=float(a[1]), s1=float(a[0]),
                )
                ot = xt
                nc.gpsimd.tensor_mul(ot[:], mt[:], rt[:])
                nc.sync.dma_start(out=y[r0:r0 + P, :], in_=ot[:])
    nc.compile()
    return nc


def _kernel_opt(x, a, c, G):
    from concourse import bass_utils

    nc = _build_opt_module(a, c, G)
    coef_arr = np.zeros((P, G + 4), np.float32)
    coef_arr[:, :G] = c[:, 1][None, :]
    coef_arr[:, G] = a[2]

    xr = np.asarray(x, np.float32).reshape(B, N_CORES, L_SH, D)
    in_maps = [
        {"x": np.ascontiguousarray(xr[:, core]).reshape(ROWS, D),
         "coef": coef_arr}
        for core in range(N_CORES)
    ]
    res = bass_utils.run_bass_kernel_spmd(nc, in_maps, core_ids=list(range(N_CORES)))
    out = np.empty((B, N_CORES, L_SH, D), np.float32)
    for core in range(N_CORES):
        out[:, core] = res.results[core]["y"].reshape(B, L_SH, D)
    return out.reshape(B, L, D)


def kernel(x, weight_numerator, weight_denominator, num_groups):
    x = np.ascontiguousarray(np.asarray(x, dtype=np.float32))
    a = np.asarray(weight_numerator, np.float32).reshape(-1)          # (6,)
    wd = np.asarray(weight_denominator, np.float32)                   # (G,4)
    G = int(num_groups)
    c = np.abs(np.concatenate([np.ones((G, 1), np.float32), wd], axis=1))

    if VARIANT == "pf" and G == G_FIXED and x.shape == (B, L, D):
        out = _kernel_pf(x, a, c)
        if out is not None:
            return out
    return _kernel_opt(x, a, c, G)


# revision 25
# speedup vs baseline: 1.8879x; 1.0095x over previous
"""KAT rational-group activation kernel for Trainium2 (Bass/Tile), 8-core SPMD.

Computes out = num(x) / den_g(x) elementwise over x:(4,4096,2048) f32, where
  num(x) = quintic (coeffs shared), den_g(x) = 1 + c1 x + ... + c4 x^4 per
  group g = channel // 256 (8 groups).

Fast path ("pf", partial-fraction): rewrite via polynomial division
  num/den = alpha*x + beta + R(x)/den(x),   deg R <= 3
then normalize with three free knobs so the device program needs only two
full custom-DVE passes per element:
  - lam_g  (host-side per-channel scale of x:  z = lam*x)
  - sig_r_g (folded into the ACT reciprocal's input scale)
  - sig2_g (host-side per-channel scale of the output)
chosen so that rho3~ = 1, rho0~ = 1, alpha~ = +/-1. Device per tile:
  1. custom DVE KAT_DEN:   den = (((c4''z+c3'')z+c2'')z+c1'')z + 1
  2. ACT Reciprocal:       r = 1/(sig_r * den)
  3. custom DVE KAT_PF_OUT: out = (((z+p2)z+p1)z+1)*r (+/-) z + b2
Host: out = sig2_g * out_dev.

Data layout: channels on partitions (host transposes x), sequence sharded
across 8 cores. I/O in fp16 (tolerance is 2e-2 relative to global max; fp16
end-to-end error measured ~7e-4). All SBUF intermediates f32.

Fallback path "opt" (previous kernel) is kept for degenerate coefficient
sets where the normalization is ill-conditioned.
"""

import numpy as np

B, L, D = 4, 4096, 2048
G_FIXED = 8
N_CORES = 8
P = 128                        # SBUF partitions
ROWS_ALL = B * L               # 16384 rows total
RW = ROWS_ALL // N_CORES       # 2048 rows per core (free dim)
NBLK = D // P                  # 16 channel blocks (partition tiles)

# legacy constants for the "opt" fallback (row-sharded layout)
L_SH = L // N_CORES
ROWS = B * L_SH
N_TILES = ROWS // P

_OPS_CACHE = {}


def _register_ops():
    """Define + register the KAT custom DVE ops (idempotent)."""
    if _OPS_CACHE:
        return _OPS_CACHE

    from concourse import dve_ops
    from concourse.dve_ops import DveOp
    from concourse.dve_spec import (
        C0, C1, C2, C3, One, Spec, Src0, Src1,
        _has_src1, _spill_c3_to_src1, lower,
    )
    from concourse.dve_uop import DveOpSpec

    # den = (((c4*z + c3)*z + c2)*z + c1)*z + 1   [C0..C2 imm, C3 -> in1 spill]
    den_body = _spill_c3_to_src1(
        (((C0 * Src0 + C1) * Src0 + C2) * Src0 + C3) * Src0 + One
    )
    den_ref = lambda in0, in1, s0, s1, imm2: (
        (((s0 * in0.astype(np.float32) + s1) * in0 + imm2) * in0
         + np.asarray(in1, np.float32).reshape(-1, 1)) * in0 + 1.0
    )

    # Q = ((a5*x + a4)*x + a3)*x + a2             [C0..C2 imm, C3 -> in1 spill]
    numq_body = _spill_c3_to_src1(
        ((C0 * Src0 + C1) * Src0 + C2) * Src0 + C3
    )
    numq_ref = lambda in0, in1, s0, s1, imm2: (
        ((s0 * in0.astype(np.float32) + s1) * in0 + imm2) * in0
        + np.asarray(in1, np.float32).reshape(-1, 1)
    )

    # M = (Q*x + a1)*x + a0                        [two full streams]
    numm_body = (Src0 * Src1 + C0) * Src1 + C1
    numm_ref = lambda in0, in1, s0, s1, imm2: (
        (in0.astype(np.float32) * in1 + s0) * in1 + s1
    )

    # out = (((z + p2)*z + p1)*z + 1)*r + z + b2   [partial-fraction tail, +z]
    pfp_body = ((((Src0 + C0) * Src0 + C1) * Src0 + One) * Src1) + Src0 + C2
    pfp_ref = lambda in0, in1, s0, s1, imm2: (
        (((in0.astype(np.float32) + s0) * in0 + s1) * in0 + 1.0) * in1
        + in0 + imm2
    )

    # out = (((z + p2)*z + p1)*z + 1)*r - z + b2   [partial-fraction tail, -z]
    pfn_body = ((((Src0 + C0) * Src0 + C1) * Src0 + One) * Src1) - Src0 + C2
    pfn_ref = lambda in0, in1, s0, s1, imm2: (
        (((in0.astype(np.float32) + s0) * in0 + s1) * in0 + 1.0) * in1
        - in0 + imm2
    )

    defs = [
        ("KAT_DEN", den_body, den_ref),
        ("KAT_NUMQ", numq_body, numq_ref),
        ("KAT_NUMM", numm_body, numm_ref),
        ("KAT_PF_OUTP", pfp_body, pfp_ref),
        ("KAT_PF_OUTN", pfn_body, pfn_ref),
    ]

    existing = {op.name for op in dve_ops.OPS}
    for name, body, ref in defs:
        if name in existing:
            _OPS_CACHE[name] = next(op for op in dve_ops.OPS if op.name == name)
            continue
        spec = Spec(body=body, reference=ref)
        row = max(dve_ops._SUB_OPCODE_FOR_NAME.values()) + 1
        assert row < 0x20, "custom DVE row field overflow"
        dve_ops._SUB_OPCODE_FOR_NAME[name] = row
        shas = {}
        for ver in ("v3", "v4"):
            uops = lower(spec, ver=ver)
            shas[ver] = DveOpSpec(
                name=name, opcode=row, uops=uops, rd1_en=_has_src1(spec)
            ).sha(ver)
        op = DveOp(name, spec, subdim=False, uops_sha=shas)
        dve_ops.OPS.append(op)
        dve_ops.CUSTOM_DVE_SPECS[name] = spec
        _OPS_CACHE[name] = op
    return _OPS_CACHE


def derive_pf_params(a, c):
    """Per-group partial-fraction constants, or None if ill-conditioned.

    a: (6,) numerator coeffs a0..a5. c: (G,5) denominator coeffs c0..c4
    (c0 == 1). Returns list of dicts per group with keys:
      lam, sig2, sig_r, rho2t, rho1t, beta2, cden (c1''..c4''), pos (bool).
    """
    a = np.asarray(a, np.float64).reshape(-1)
    c = np.asarray(c, np.float64)
    G = c.shape[0]
    out = []
    for g in range(G):
        cg = c[g]
        if abs(cg[4]) < 1e-12:
            return None
        q, r = np.polydiv(a[::-1], cg[::-1])
        if len(q) != 2:
            return None
        alpha, beta = q[0], q[1]
        R = r[::-1]
        R = np.pad(R, (0, 4 - len(R)))
        rho0, rho1, rho2, rho3 = R
        if abs(rho0) < 1e-10 or abs(rho3) < 1e-12 or abs(alpha) < 1e-10:
            return None
        lam = np.cbrt(rho3 / rho0)
        if not (2.0**-6 < abs(lam) < 2.0**6):
            return None
        sig2 = alpha / lam          # alpha~ = +1 variant
        sig_r = sig2 / rho0
        pos = True
        if sig_r < 0:               # flip to alpha~ = -1 so sig_r > 0
            sig2, sig_r, pos = -sig2, -sig_r, False
        if not (2.0**-9 < abs(sig2) < 2.0**14):
            return None
        rho2t = rho2 / (rho0 * lam * lam)
        rho1t = rho1 / (rho0 * lam)
        beta2 = beta / sig2
        # den coeffs in z = lam*x coordinates: c_k'' = c_k / lam^k
        cden = cg[1:5] / lam ** np.arange(1, 5)
        vals = [lam, sig2, sig_r, rho2t, rho1t, beta2, *cden]
        if not all(np.isfinite(vals)):
            return None
        d = dict(lam=lam, sig2=sig2, sig_r=sig_r, rho2t=rho2t,
                 rho1t=rho1t, beta2=beta2, cden=cden, pos=pos, fac=None)
        # factored den for the ACT/Pool offload path:
        #   den'' = c4''*((z+h1)^2+k1)*((z+h2)^2+k2)
        d["fac"] = _factor_quartic(cden, lam)
        out.append(d)
    return out


def _factor_quartic(cden, lam):
    """Factor 1 + c1''z + ... + c4''z^4 into c4''*(z^2+p1z+q1)(z^2+p2z+q2).
    Returns (h1,k1,h2,k2) with quadratic = (z+h)^2 + k, or None."""
    try:
        roots = np.roots([cden[3], cden[2], cden[1], cden[0], 1.0])
    except Exception:
        return None
    if len(roots) != 4:
        return None
    cplx = [r for r in roots if abs(r.imag) > 1e-9]
    reals = sorted(r.real for r in roots if abs(r.imag) <= 1e-9)
    quads = []
    used = set()
    for i, z1 in enumerate(cplx):
        if i in used:
            continue
        for j in range(i + 1, len(cplx)):
            if j not in used and abs(np.conj(z1) - cplx[j]) < 1e-6 * max(1, abs(z1)):
                quads.append((-2 * z1.real, abs(z1) ** 2))
                used.add(i)
                used.add(j)
                break
    while len(reals) >= 2:
        r1 = reals.pop(0)
        r2 = reals.pop(-1)
        quads.append((-(r1 + r2), r1 * r2))
    if len(quads) != 2:
        return None
    (p1, q1), (p2, q2) = quads
    h1, k1 = p1 / 2, q1 - p1 * p1 / 4
    h2, k2 = p2 / 2, q2 - p2 * p2 / 4
    if not all(np.isfinite([h1, k1, h2, k2])):
        return None
    # validate on the data range
    zz = np.linspace(-5.8 * abs(lam), 5.8 * abs(lam), 4001)
    den_h = (((cden[3] * zz + cden[2]) * zz + cden[1]) * zz + cden[0]) * zz + 1.0
    den_f = cden[3] * ((zz + h1) ** 2 + k1) * ((zz + h2) ** 2 + k2)
    if np.abs(den_f - den_h).max() > 1e-5 * np.abs(den_h).min():
        return None
    return (float(h1), float(k1), float(h2), float(k2))


VARIANT = "pf"  # "pf" fast path; "opt" fallback
# full blocks whose den is computed on ACT/Pool instead of DVE
OFFLOAD_BLOCKS = (2, 5, 8, 11)
OFF_DELAY = 3
RECIP_PRIO = 0
RR_ON_DVE = False
STT_DEN = False
XIN_PRIO = 0
PREFETCH = 3
OUT_ENG = 'sync'
OFF_FIRST = False
HEAD_SPLITS = [256, 512, 512, 768]
TAIL_SPLITS = [768, 512, 512, 256]


def _build_module(a, c, G, variant=None):
    """Trace the per-core Bass module. a:(6,) numerator, c:(G,5) |den| coeffs."""
    variant = VARIANT if variant is None else variant
    if variant == "pf":
        params = derive_pf_params(a, c)
        if params is not None and G == G_FIXED:
            return _build_pf(params)
        variant = "opt"
    return _build_opt_module(a, c, G)


def _build_pf(params):
    """Partial-fraction module: [2048 ch, 2048 rows] fp16 in/out per core.

    Channels on partitions; each of the 16 partition tiles lies in a single
    group, so all per-group constants are instruction immediates.
    """
    import concourse.bacc as bacc
    import concourse.mybir as mybir
    from concourse.tile import TileContext

    ops = _register_ops()
    f32 = mybir.dt.float32
    f16 = mybir.dt.float16
    imm = lambda v: mybir.ImmediateValue(dtype=mybir.dt.float32, value=float(v))

    nc = bacc.Bacc("TRN2", target_bir_lowering=False)
    x = nc.dram_tensor("x", (D, RW), f16, kind="ExternalInput")
    coef = nc.dram_tensor("coef", (P, 3 * NBLK), f32, kind="ExternalInput")
    y = nc.dram_tensor("y", (D, RW), f16, kind="ExternalOutput")

    # graduated pieces: small at the head (fast pipeline fill) and at the
    # tail (short serial drain chain); full-size tiles mid-stream.
    pieces = []                    # (channel block, row start, row count)
    for b in range(NBLK):
        if b == 0:
            splits = HEAD_SPLITS
        elif b == NBLK - 1:
            splits = TAIL_SPLITS
        else:
            splits = [RW]
        r0 = 0
        for n in splits:
            pieces.append((b, r0, n))
            r0 += n

    def group_of(b):
        return b * P // (D // G_FIXED)

    def order_pieces(pieces, offload):
        if not OFF_FIRST:
            return pieces
        head = [p for p in pieces if p[0] == 0]
        offp = [p for p in pieces if p[0] in offload]
        rest = [p for p in pieces if p[0] != 0 and p[0] not in offload]
        return head + offp + rest

    # den offload (ACT Squares + Pool STT) for these full blocks, when the
    # group's quartic factorization is available
    def fac_ok(g):
        f = params[g]["fac"]
        return f is not None and f[1] > 1e-3 and f[3] > 1e-3
    offload = {b for b in OFFLOAD_BLOCKS if 0 < b < NBLK - 1 and fac_ok(group_of(b))}
    AF = mybir.ActivationFunctionType
    ALU = mybir.AluOpType

    full = RW

    def OUT_ENGINE():
        return getattr(nc, OUT_ENG)

    with TileContext(nc) as tc:
        with tc.tile_pool(name="const", bufs=1) as cpool, \
             tc.tile_pool(name="x", bufs=1) as xpool, \
             tc.tile_pool(name="den", bufs=1) as dpool, \
             tc.tile_pool(name="rec", bufs=1) as rpool, \
             tc.tile_pool(name="out", bufs=1) as opool, \
             tc.tile_pool(name="sqa", bufs=1) as apool, \
             tc.tile_pool(name="sqb", bufs=1) as bpool, \
             tc.tile_pool(name="w1", bufs=1) as wpool:
            ct = cpool.tile([P, 3 * NBLK], f32)
            nc.scalar.dma_start(out=ct[:], in_=coef[:, :])

            off_state = {}   # b -> (xt, w1_tile)

            def emit_off_den(b, n, r0, xt):
                """den offload: ACT Squares + ACT per-factor reciprocals
                (k folded into recip bias), Pool multiplies the factors.
                Leaves rr = 1/(sig_r*den) ready for the c2 tail."""
                g = group_of(b)
                pg = params[g]
                h1, k1, h2, k2 = pg["fac"]
                sc = pg["sig_r"] * pg["cden"][3]
                at = apool.tile([P, n], f16, tag="a", bufs=2)
                nc.scalar.activation(at[:], xt[:], AF.Square,
                                     bias=ct[:, NBLK + 2 * b:NBLK + 2 * b + 1])
                bt = bpool.tile([P, n], f16, tag="b", bufs=2)
                nc.scalar.activation(bt[:], xt[:], AF.Square,
                                     bias=ct[:, NBLK + 2 * b + 1:NBLK + 2 * b + 2])
                r1 = apool.tile([P, n], f16, tag="r1", bufs=2)
                nc.scalar.add_instruction(
                    mybir.InstActivation(
                        name=nc.get_next_instruction_name(),
                        func=mybir.ActivationFunctionType.Reciprocal,
                        ins=[nc.scalar.lower_ap(at[:]),
                             imm(k1), imm(1.0), imm(0.0)],
                        outs=[nc.scalar.lower_ap(r1[:])],
                    )
                )
                r2 = bpool.tile([P, n], f16, tag="r2", bufs=2)
                nc.scalar.add_instruction(
                    mybir.InstActivation(
                        name=nc.get_next_instruction_name(),
                        func=mybir.ActivationFunctionType.Reciprocal,
                        ins=[nc.scalar.lower_ap(bt[:]),
                             imm(sc * k2), imm(sc), imm(0.0)],
                        outs=[nc.scalar.lower_ap(r2[:])],
                    )
                )
                rr = wpool.tile([P, n], f16, tag="rr", bufs=4)
                if RR_ON_DVE:
                    nc.vector.tensor_mul(rr[:], r1[:], r2[:])
                else:
                    nc.gpsimd.tensor_mul(rr[:], r1[:], r2[:])
                off_state[b] = (xt, rr)

            def emit_tail(b, n, r0, xt, dent, rscale, rt=None, rbias=0.0):
                g = group_of(b)
                pg = params[g]
                if rt is None:
                    rt = rpool.tile([P, n], f32, tag=f"r{n}",
                                    bufs=4 if n == full else 2)
                    nc.scalar.add_instruction(
                        mybir.InstActivation(
                            name=nc.get_next_instruction_name(),
                            func=mybir.ActivationFunctionType.Reciprocal,
                            ins=[nc.scalar.lower_ap(dent[:]),
                                 imm(rbias), imm(rscale), imm(0.0)],
                            outs=[nc.scalar.lower_ap(rt[:])],
                        )
                    )
                ot = opool.tile([P, n], f16, tag=f"o{n}",
                                bufs=3 if n == full else 2)
                nc.vector._custom_dve(
                    ops["KAT_PF_OUTP" if pg["pos"] else "KAT_PF_OUTN"],
                    out=ot[:], in0=xt[:], in1=rt[:],
                    s0=float(pg["rho2t"]), s1=float(pg["rho1t"]),
                    imm2=float(pg["beta2"]),
                )
                OUT_ENGINE().dma_start(out=y[b * P:(b + 1) * P, r0:r0 + n], in_=ot[:])

            def emit_main(b, n, r0, xt):
                g = group_of(b)
                pg = params[g]
                c1pp, c2pp, c3pp, c4pp = [float(v) for v in pg["cden"]]
                if STT_DEN:
                    # monic-quartic prefix via 3 fp16 STT ops (4x DVE mode);
                    # den = c4''*v3 + 1 folds into the reciprocal's scale+bias
                    v1 = dpool.tile([P, n], f16, tag=f"v1{n}", bufs=2)
                    nc.vector.scalar_tensor_tensor(
                        out=v1[:], in0=xt[:], scalar=c3pp / c4pp, in1=xt[:],
                        op0=ALU.add, op1=ALU.mult)
                    v2 = dpool.tile([P, n], f16, tag=f"v2{n}", bufs=2)
                    nc.vector.scalar_tensor_tensor(
                        out=v2[:], in0=v1[:], scalar=c2pp / c4pp, in1=xt[:],
                        op0=ALU.add, op1=ALU.mult)
                    dent = dpool.tile([P, n], f16, tag=f"v3{n}", bufs=3)
                    nc.vector.scalar_tensor_tensor(
                        out=dent[:], in0=v2[:], scalar=c1pp / c4pp, in1=xt[:],
                        op0=ALU.add, op1=ALU.mult)
                    sc = pg["sig_r"] * c4pp
                    emit_tail(b, n, r0, xt, dent, sc, rbias=pg["sig_r"])
                else:
                    dent = dpool.tile([P, n], f32, tag=f"d{n}",
                                      bufs=3 if n == full else 2)
                    nc.vector._custom_dve(
                        ops["KAT_DEN"],
                        out=dent[:], in0=xt[:], in1=ct[:, b:b + 1],
                        s0=c4pp, s1=c3pp, imm2=c2pp,
                    )
                    emit_tail(b, n, r0, xt, dent, pg["sig_r"])

            def pop_tail(ob):
                xt, rr = off_state.pop(ob)
                emit_tail(ob, RW, 0, xt, None, 0.0, rt=rr)

            xts = {}

            def emit_load(i):
                b, r0, n = pieces[i]
                if b in offload:
                    xt = xpool.tile([P, n], f16, tag="xo", bufs=5, name=f"xo{i}")
                else:
                    xt = xpool.tile([P, n], f16, tag=f"x{n}", name=f"xi{i}",
                                    bufs=5 if n == full else 4)
                nc.sync.dma_start(out=xt[:], in_=x[b * P:(b + 1) * P, r0:r0 + n])
                xts[i] = xt

            pieces = order_pieces(pieces, offload)
            for i in range(min(PREFETCH, len(pieces))):
                emit_load(i)
            pending = []          # (block, emit piece-index)
            for idx, (b, r0, n) in enumerate(pieces):
                if idx + PREFETCH < len(pieces):
                    emit_load(idx + PREFETCH)
                # pop deferred tails once their Pool chain is ~OFF_DELAY
                # pieces old, so they never trail the graduated drain pieces
                while pending and idx - pending[0][1] >= OFF_DELAY:
                    pop_tail(pending.pop(0)[0])
                if b in offload:
                    emit_off_den(b, n, r0, xts.pop(idx))
                    pending.append((b, idx))
                    continue
                emit_main(b, r0=r0, n=n, xt=xts.pop(idx))
            for ob, _ in pending:
                pop_tail(ob)
    nc.compile()
    return nc


def _kernel_pf(x, a, c):
    """Fast path driver. x:(B,L,D) f32. Returns (B,L,D) f32 or None."""
    from concourse import bass_utils

    params = derive_pf_params(a, c)
    if params is None:
        return None
    nc = _build_pf(params)

    Wg = D // G_FIXED
    lam_ch = np.repeat([p["lam"] for p in params], Wg)      # (D,)
    sig2_ch = np.repeat([p["sig2"] for p in params], Wg)    # (D,)

    # host: z = lam * x, transposed to [D, B*L], fp16
    x2 = np.asarray(x, np.float32).reshape(ROWS_ALL, D)
    zt = (x2.T * lam_ch[:, None].astype(np.float32)).astype(np.float16)

    coef_arr = np.zeros((P, 3 * NBLK), np.float32)
    for b in range(NBLK):
        g = b * P // Wg
        coef_arr[:, b] = params[g]["cden"][0]   # c1'' spilled via in1
        if params[g]["fac"] is not None:
            h1, k1, h2, k2 = params[g]["fac"]
            coef_arr[:, NBLK + 2 * b] = h1
            coef_arr[:, NBLK + 2 * b + 1] = h2

    in_maps = [
        {"x": np.ascontiguousarray(zt[:, core * RW:(core + 1) * RW]),
         "coef": coef_arr}
        for core in range(N_CORES)
    ]
    res = bass_utils.run_bass_kernel_spmd(nc, in_maps, core_ids=list(range(N_CORES)))

    out2 = np.empty((ROWS_ALL, D), np.float32)
    s32 = sig2_ch.astype(np.float32)[:, None]
    for core in range(N_CORES):
        ys = res.results[core]["y"]             # [D, RW] f16
        out2[core * RW:(core + 1) * RW, :] = (ys.astype(np.float32) * s32).T
    return out2.reshape(B, L, D)


# ---------------------------------------------------------------------------
# Fallback "opt" path (previous kernel): row-sharded f32, den/recip/num/mul.
# ---------------------------------------------------------------------------

def _build_opt_module(a, c, G):
    import concourse.bacc as bacc
    import concourse.mybir as mybir
    from concourse.tile import TileContext

    ops = _register_ops()
    f32 = mybir.dt.float32
    W = D // G
    imm = lambda v: mybir.ImmediateValue(dtype=mybir.dt.float32, value=v)

    nc = bacc.Bacc("TRN2", target_bir_lowering=False)
    x = nc.dram_tensor("x", (ROWS, D), f32, kind="ExternalInput")
    coef = nc.dram_tensor("coef", (P, G + 4), f32, kind="ExternalInput")
    y = nc.dram_tensor("y", (ROWS, D), f32, kind="ExternalOutput")

    with TileContext(nc) as tc:
        with tc.tile_pool(name="const", bufs=1) as cpool, \
             tc.tile_pool(name="xo", bufs=4) as xpool, \
             tc.tile_pool(name="work", bufs=3) as pool:
            ct = cpool.tile([P, G + 4], f32)
            nc.sync.dma_start(out=ct[:], in_=coef[:, :])
            for i in range(N_TILES):
                r0 = i * P
                xt = xpool.tile([P, D], f32, tag="x")
                nc.sync.dma_start(out=xt[:], in_=x[r0:r0 + P, :])
                dent = pool.tile([P, D], f32, tag="den")
                for g in range(G):
                    sl = slice(g * W, (g + 1) * W)
                    nc.vector._custom_dve(
                        ops["KAT_DEN"],
                        out=dent[:, sl], in0=xt[:, sl], in1=ct[:, g:g + 1],
                        s0=float(c[g, 4]), s1=float(c[g, 3]), imm2=float(c[g, 2]),
                    )
                rt = pool.tile([P, D], f32, tag="r")
                nc.scalar.add_instruction(
                    mybir.InstActivation(
                        name=nc.get_next_instruction_name(),
                        func=mybir.ActivationFunctionType.Reciprocal,
                        ins=[nc.scalar.lower_ap(dent[:]),
                             imm(0.0), imm(1.0), imm(0.0)],
                        outs=[nc.scalar.lower_ap(rt[:])],
                    )
                )
                qt = pool.tile([P, D], f32, tag="q")
                nc.vector._custom_dve(
                    ops["KAT_NUMQ"],
                    out=qt[:], in0=xt[:], in1=ct[:, G:G + 1],
                    s0=float(a[5]), s1=float(a[4]), imm2=float(a[3]),
                )
                mt = pool.tile([P, D], f32, tag="m")
                nc.vector._custom_dve(
                    ops["KAT_NUMM"],
                    out=mt[:], in0=qt[:], in1=xt[:],
                    s0=float(a[1]), s1=float(a[0]),
                )
                ot = xt
                nc.gpsimd.tensor_mul(ot[:], mt[:], rt[:])
                nc.sync.dma_start(out=y[r0:r0 + P, :], in_=ot[:])
    nc.compile()
    return nc


def _kernel_opt(x, a, c, G):
    from concourse import bass_utils

    nc = _build_opt_module(a, c, G)
    coef_arr = np.zeros((P, G + 4), np.float32)
    coef_arr[:, :G] = c[:, 1][None, :]
    coef_arr[:, G] = a[2]

    xr = np.asarray(x, np.float32).reshape(B, N_CORES, L_SH, D)
    in_maps = [
        {"x": np.ascontiguousarray(xr[:, core]).reshape(ROWS, D),
         "coef": coef_arr}
        for core in range(N_CORES)
    ]
    res = bass_utils.run_bass_kernel_spmd(nc, in_maps, core_ids=list(range(N_CORES)))
    out = np.empty((B, N_CORES, L_SH, D), np.float32)
    for core in range(N_CORES):
        out[:, core] = res.results[core]["y"].reshape(B, L_SH, D)
    return out.reshape(B, L, D)


def kernel(x, weight_numerator, weight_denominator, num_groups):
    x = np.ascontiguousarray(np.asarray(x, dtype=np.float32))
    a = np.asarray(weight_numerator, np.float32).reshape(-1)          # (6,)
    wd = np.asarray(weight_denominator, np.float32)                   # (G,4)
    G = int(num_groups)
    c = np.abs(np.concatenate([np.ones((G, 1), np.float32), wd], axis=1))

    if VARIANT == "pf" and G == G_FIXED and x.shape == (B, L, D):
        out = _kernel_pf(x, a, c)
        if out is not None:
            return out
    return _kernel_opt(x, a, c, G)


# revision 26
# speedup vs baseline: 1.9077x; 1.0105x over previous
"""KAT rational-group activation kernel for Trainium2 (Bass/Tile), 8-core SPMD.

Computes out = num(x) / den_g(x) elementwise over x:(4,4096,2048) f32, where
  num(x) = quintic (coeffs shared), den_g(x) = 1 + c1 x + ... + c4 x^4 per
  group g = channel // 256 (8 groups).

Fast path ("pf", partial-fraction): rewrite via polynomial division
  num/den = alpha*x + beta + R(x)/den(x),   deg R <= 3
then normalize with three free knobs so the device program needs only two
full custom-DVE passes per element:
  - lam_g  (host-side per-channel scale of x:  z = lam*x)
  - sig_r_g (folded into the ACT reciprocal's input scale)
  - sig2_g (host-side per-channel scale of the output)
chosen so that rho3~ = 1, rho0~ = 1, alpha~ = +/-1. Device per tile:
  1. custom DVE KAT_DEN:   den = (((c4''z+c3'')z+c2'')z+c1'')z + 1
  2. ACT Reciprocal:       r = 1/(sig_r * den)
  3. custom DVE KAT_PF_OUT: out = (((z+p2)z+p1)z+1)*r (+/-) z + b2
Host: out = sig2_g * out_dev.

Data layout: channels on partitions (host transposes x), sequence sharded
across 8 cores. I/O in fp16 (tolerance is 2e-2 relative to global max; fp16
end-to-end error measured ~7e-4). All SBUF intermediates f32.

Fallback path "opt" (previous kernel) is kept for degenerate coefficient
sets where the normalization is ill-conditioned.
"""

import numpy as np

B, L, D = 4, 4096, 2048
G_FIXED = 8
N_CORES = 8
P = 128                        # SBUF partitions
ROWS_ALL = B * L               # 16384 rows total
RW = ROWS_ALL // N_CORES       # 2048 rows per core (free dim)
NBLK = D // P                  # 16 channel blocks (partition tiles)

# legacy constants for the "opt" fallback (row-sharded layout)
L_SH = L // N_CORES
ROWS = B * L_SH
N_TILES = ROWS // P

_OPS_CACHE = {}


def _register_ops():
    """Define + register the KAT custom DVE ops (idempotent)."""
    if _OPS_CACHE:
        return _OPS_CACHE

    from concourse import dve_ops
    from concourse.dve_ops import DveOp
    from concourse.dve_spec import (
        C0, C1, C2, C3, One, Spec, Src0, Src1,
        _has_src1, _spill_c3_to_src1, lower,
    )
    from concourse.dve_uop import DveOpSpec

    # den = (((c4*z + c3)*z + c2)*z + c1)*z + 1   [C0..C2 imm, C3 -> in1 spill]
    den_body = _spill_c3_to_src1(
        (((C0 * Src0 + C1) * Src0 + C2) * Src0 + C3) * Src0 + One
    )
    den_ref = lambda in0, in1, s0, s1, imm2: (
        (((s0 * in0.astype(np.float32) + s1) * in0 + imm2) * in0
         + np.asarray(in1, np.float32).reshape(-1, 1)) * in0 + 1.0
    )

    # Q = ((a5*x + a4)*x + a3)*x + a2             [C0..C2 imm, C3 -> in1 spill]
    numq_body = _spill_c3_to_src1(
        ((C0 * Src0 + C1) * Src0 + C2) * Src0 + C3
    )
    numq_ref = lambda in0, in1, s0, s1, imm2: (
        ((s0 * in0.astype(np.float32) + s1) * in0 + imm2) * in0
        + np.asarray(in1, np.float32).reshape(-1, 1)
    )

    # M = (Q*x + a1)*x + a0                        [two full streams]
    numm_body = (Src0 * Src1 + C0) * Src1 + C1
    numm_ref = lambda in0, in1, s0, s1, imm2: (
        (in0.astype(np.float32) * in1 + s0) * in1 + s1
    )

    # out = (((z + p2)*z + p1)*z + 1)*r + z + b2   [partial-fraction tail, +z]
    pfp_body = ((((Src0 + C0) * Src0 + C1) * Src0 + One) * Src1) + Src0 + C2
    pfp_ref = lambda in0, in1, s0, s1, imm2: (
        (((in0.astype(np.float32) + s0) * in0 + s1) * in0 + 1.0) * in1
        + in0 + imm2
    )

    # out = (((z + p2)*z + p1)*z + 1)*r - z + b2   [partial-fraction tail, -z]
    pfn_body = ((((Src0 + C0) * Src0 + C1) * Src0 + One) * Src1) - Src0 + C2
    pfn_ref = lambda in0, in1, s0, s1, imm2: (
        (((in0.astype(np.float32) + s0) * in0 + s1) * in0 + 1.0) * in1
        - in0 + imm2
    )

    defs = [
        ("KAT_DEN", den_body, den_ref),
        ("KAT_NUMQ", numq_body, numq_ref),
        ("KAT_NUMM", numm_body, numm_ref),
        ("KAT_PF_OUTP", pfp_body, pfp_ref),
        ("KAT_PF_OUTN", pfn_body, pfn_ref),
    ]

    existing = {op.name for op in dve_ops.OPS}
    for name, body, ref in defs:
        if name in existing:
            _OPS_CACHE[name] = next(op for op in dve_ops.OPS if op.name == name)
            continue
        spec = Spec(body=body, reference=ref)
        row = max(dve_ops._SUB_OPCODE_FOR_NAME.values()) + 1
        assert row < 0x20, "custom DVE row field overflow"
        dve_ops._SUB_OPCODE_FOR_NAME[name] = row
        shas = {}
        for ver in ("v3", "v4"):
            uops = lower(spec, ver=ver)
            shas[ver] = DveOpSpec(
                name=name, opcode=row, uops=uops, rd1_en=_has_src1(spec)
            ).sha(ver)
        op = DveOp(name, spec, subdim=False, uops_sha=shas)
        dve_ops.OPS.append(op)
        dve_ops.CUSTOM_DVE_SPECS[name] = spec
        _OPS_CACHE[name] = op
    return _OPS_CACHE


def derive_pf_params(a, c):
    """Per-group partial-fraction constants, or None if ill-conditioned.

    a: (6,) numerator coeffs a0..a5. c: (G,5) denominator coeffs c0..c4
    (c0 == 1). Returns list of dicts per group with keys:
      lam, sig2, sig_r, rho2t, rho1t, beta2, cden (c1''..c4''), pos (bool).
    """
    a = np.asarray(a, np.float64).reshape(-1)
    c = np.asarray(c, np.float64)
    G = c.shape[0]
    out = []
    for g in range(G):
        cg = c[g]
        if abs(cg[4]) < 1e-12:
            return None
        q, r = np.polydiv(a[::-1], cg[::-1])
        if len(q) != 2:
            return None
        alpha, beta = q[0], q[1]
        R = r[::-1]
        R = np.pad(R, (0, 4 - len(R)))
        rho0, rho1, rho2, rho3 = R
        if abs(rho0) < 1e-10 or abs(rho3) < 1e-12 or abs(alpha) < 1e-10:
            return None
        lam = np.cbrt(rho3 / rho0)
        if not (2.0**-6 < abs(lam) < 2.0**6):
            return None
        sig2 = alpha / lam          # alpha~ = +1 variant
        sig_r = sig2 / rho0
        pos = True
        if sig_r < 0:               # flip to alpha~ = -1 so sig_r > 0
            sig2, sig_r, pos = -sig2, -sig_r, False
        if not (2.0**-9 < abs(sig2) < 2.0**14):
            return None
        rho2t = rho2 / (rho0 * lam * lam)
        rho1t = rho1 / (rho0 * lam)
        beta2 = beta / sig2
        # den coeffs in z = lam*x coordinates: c_k'' = c_k / lam^k
        cden = cg[1:5] / lam ** np.arange(1, 5)
        vals = [lam, sig2, sig_r, rho2t, rho1t, beta2, *cden]
        if not all(np.isfinite(vals)):
            return None
        d = dict(lam=lam, sig2=sig2, sig_r=sig_r, rho2t=rho2t,
                 rho1t=rho1t, beta2=beta2, cden=cden, pos=pos, fac=None)
        # factored den for the ACT/Pool offload path:
        #   den'' = c4''*((z+h1)^2+k1)*((z+h2)^2+k2)
        d["fac"] = _factor_quartic(cden, lam)
        out.append(d)
    return out


def _factor_quartic(cden, lam):
    """Factor 1 + c1''z + ... + c4''z^4 into c4''*(z^2+p1z+q1)(z^2+p2z+q2).
    Returns (h1,k1,h2,k2) with quadratic = (z+h)^2 + k, or None."""
    try:
        roots = np.roots([cden[3], cden[2], cden[1], cden[0], 1.0])
    except Exception:
        return None
    if len(roots) != 4:
        return None
    cplx = [r for r in roots if abs(r.imag) > 1e-9]
    reals = sorted(r.real for r in roots if abs(r.imag) <= 1e-9)
    quads = []
    used = set()
    for i, z1 in enumerate(cplx):
        if i in used:
            continue
        for j in range(i + 1, len(cplx)):
            if j not in used and abs(np.conj(z1) - cplx[j]) < 1e-6 * max(1, abs(z1)):
                quads.append((-2 * z1.real, abs(z1) ** 2))
                used.add(i)
                used.add(j)
                break
    while len(reals) >= 2:
        r1 = reals.pop(0)
        r2 = reals.pop(-1)
        quads.append((-(r1 + r2), r1 * r2))
    if len(quads) != 2:
        return None
    (p1, q1), (p2, q2) = quads
    h1, k1 = p1 / 2, q1 - p1 * p1 / 4
    h2, k2 = p2 / 2, q2 - p2 * p2 / 4
    if not all(np.isfinite([h1, k1, h2, k2])):
        return None
    # validate on the data range
    zz = np.linspace(-5.8 * abs(lam), 5.8 * abs(lam), 4001)
    den_h = (((cden[3] * zz + cden[2]) * zz + cden[1]) * zz + cden[0]) * zz + 1.0
    den_f = cden[3] * ((zz + h1) ** 2 + k1) * ((zz + h2) ** 2 + k2)
    if np.abs(den_f - den_h).max() > 1e-5 * np.abs(den_h).min():
        return None
    return (float(h1), float(k1), float(h2), float(k2))


VARIANT = "pf"  # "pf" fast path; "opt" fallback
# full blocks whose den is computed on ACT/Pool instead of DVE
OFFLOAD_BLOCKS = (3, 6, 9, 12)
OFF_DELAY = 3
RECIP_PRIO = 0
RR_ON_DVE = False
STT_DEN = False
XIN_PRIO = 0
PREFETCH = 3
OUT_ENG = 'sync'
OFF_FIRST = False
HEAD_SPLITS = [256, 512, 512, 768]
TAIL_SPLITS = [768, 512, 512, 256]


def _build_module(a, c, G, variant=None):
    """Trace the per-core Bass module. a:(6,) numerator, c:(G,5) |den| coeffs."""
    variant = VARIANT if variant is None else variant
    if variant == "pf":
        params = derive_pf_params(a, c)
        if params is not None and G == G_FIXED:
            return _build_pf(params)
        variant = "opt"
    return _build_opt_module(a, c, G)


def _build_pf(params):
    """Partial-fraction module: [2048 ch, 2048 rows] fp16 in/out per core.

    Channels on partitions; each of the 16 partition tiles lies in a single
    group, so all per-group constants are instruction immediates.
    """
    import concourse.bacc as bacc
    import concourse.mybir as mybir
    from concourse.tile import TileContext

    ops = _register_ops()
    f32 = mybir.dt.float32
    f16 = mybir.dt.float16
    imm = lambda v: mybir.ImmediateValue(dtype=mybir.dt.float32, value=float(v))

    nc = bacc.Bacc("TRN2", target_bir_lowering=False)
    x = nc.dram_tensor("x", (D, RW), f16, kind="ExternalInput")
    coef = nc.dram_tensor("coef", (P, 3 * NBLK), f32, kind="ExternalInput")
    y = nc.dram_tensor("y", (D, RW), f16, kind="ExternalOutput")

    # graduated pieces: small at the head (fast pipeline fill) and at the
    # tail (short serial drain chain); full-size tiles mid-stream.
    pieces = []                    # (channel block, row start, row count)
    for b in range(NBLK):
        if b == 0:
            splits = HEAD_SPLITS
        elif b == NBLK - 1:
            splits = TAIL_SPLITS
        else:
            splits = [RW]
        r0 = 0
        for n in splits:
            pieces.append((b, r0, n))
            r0 += n

    def group_of(b):
        return b * P // (D // G_FIXED)

    def order_pieces(pieces, offload):
        if not OFF_FIRST:
            return pieces
        head = [p for p in pieces if p[0] == 0]
        offp = [p for p in pieces if p[0] in offload]
        rest = [p for p in pieces if p[0] != 0 and p[0] not in offload]
        return head + offp + rest

    # den offload (ACT Squares + Pool STT) for these full blocks, when the
    # group's quartic factorization is available
    def fac_ok(g):
        f = params[g]["fac"]
        return f is not None and f[1] > 1e-3 and f[3] > 1e-3
    offload = {b for b in OFFLOAD_BLOCKS if 0 < b < NBLK - 1 and fac_ok(group_of(b))}
    AF = mybir.ActivationFunctionType
    ALU = mybir.AluOpType

    full = RW

    def OUT_ENGINE():
        return getattr(nc, OUT_ENG)

    with TileContext(nc) as tc:
        with tc.tile_pool(name="const", bufs=1) as cpool, \
             tc.tile_pool(name="x", bufs=1) as xpool, \
             tc.tile_pool(name="den", bufs=1) as dpool, \
             tc.tile_pool(name="rec", bufs=1) as rpool, \
             tc.tile_pool(name="out", bufs=1) as opool, \
             tc.tile_pool(name="sqa", bufs=1) as apool, \
             tc.tile_pool(name="sqb", bufs=1) as bpool, \
             tc.tile_pool(name="w1", bufs=1) as wpool:
            ct = cpool.tile([P, 3 * NBLK], f32)
            nc.scalar.dma_start(out=ct[:], in_=coef[:, :])

            off_state = {}   # b -> (xt, w1_tile)

            def emit_off_den(b, n, r0, xt):
                """den offload: ACT Squares + ACT per-factor reciprocals
                (k folded into recip bias), Pool multiplies the factors.
                Leaves rr = 1/(sig_r*den) ready for the c2 tail."""
                g = group_of(b)
                pg = params[g]
                h1, k1, h2, k2 = pg["fac"]
                sc = pg["sig_r"] * pg["cden"][3]
                at = apool.tile([P, n], f16, tag="a", bufs=2)
                nc.scalar.activation(at[:], xt[:], AF.Square,
                                     bias=ct[:, NBLK + 2 * b:NBLK + 2 * b + 1])
                bt = bpool.tile([P, n], f16, tag="b", bufs=2)
                nc.scalar.activation(bt[:], xt[:], AF.Square,
                                     bias=ct[:, NBLK + 2 * b + 1:NBLK + 2 * b + 2])
                r1 = apool.tile([P, n], f16, tag="r1", bufs=2)
                nc.scalar.add_instruction(
                    mybir.InstActivation(
                        name=nc.get_next_instruction_name(),
                        func=mybir.ActivationFunctionType.Reciprocal,
                        ins=[nc.scalar.lower_ap(at[:]),
                             imm(k1), imm(1.0), imm(0.0)],
                        outs=[nc.scalar.lower_ap(r1[:])],
                    )
                )
                r2 = bpool.tile([P, n], f16, tag="r2", bufs=2)
                nc.scalar.add_instruction(
                    mybir.InstActivation(
                        name=nc.get_next_instruction_name(),
                        func=mybir.ActivationFunctionType.Reciprocal,
                        ins=[nc.scalar.lower_ap(bt[:]),
                             imm(sc * k2), imm(sc), imm(0.0)],
                        outs=[nc.scalar.lower_ap(r2[:])],
                    )
                )
                rr = wpool.tile([P, n], f16, tag="rr", bufs=4)
                if RR_ON_DVE:
                    nc.vector.tensor_mul(rr[:], r1[:], r2[:])
                else:
                    nc.gpsimd.tensor_mul(rr[:], r1[:], r2[:])
                off_state[b] = (xt, rr)

            def emit_tail(b, n, r0, xt, dent, rscale, rt=None, rbias=0.0):
                g = group_of(b)
                pg = params[g]
                if rt is None:
                    rt = rpool.tile([P, n], f32, tag=f"r{n}",
                                    bufs=4 if n == full else 2)
                    nc.scalar.add_instruction(
                        mybir.InstActivation(
                            name=nc.get_next_instruction_name(),
                            func=mybir.ActivationFunctionType.Reciprocal,
                            ins=[nc.scalar.lower_ap(dent[:]),
                                 imm(rbias), imm(rscale), imm(0.0)],
                            outs=[nc.scalar.lower_ap(rt[:])],
                        )
                    )
                ot = opool.tile([P, n], f16, tag=f"o{n}",
                                bufs=3 if n == full else 2)
                nc.vector._custom_dve(
                    ops["KAT_PF_OUTP" if pg["pos"] else "KAT_PF_OUTN"],
                    out=ot[:], in0=xt[:], in1=rt[:],
                    s0=float(pg["rho2t"]), s1=float(pg["rho1t"]),
                    imm2=float(pg["beta2"]),
                )
                OUT_ENGINE().dma_start(out=y[b * P:(b + 1) * P, r0:r0 + n], in_=ot[:])

            def emit_main(b, n, r0, xt):
                g = group_of(b)
                pg = params[g]
                c1pp, c2pp, c3pp, c4pp = [float(v) for v in pg["cden"]]
                if STT_DEN:
                    # monic-quartic prefix via 3 fp16 STT ops (4x DVE mode);
                    # den = c4''*v3 + 1 folds into the reciprocal's scale+bias
                    v1 = dpool.tile([P, n], f16, tag=f"v1{n}", bufs=2)
                    nc.vector.scalar_tensor_tensor(
                        out=v1[:], in0=xt[:], scalar=c3pp / c4pp, in1=xt[:],
                        op0=ALU.add, op1=ALU.mult)
                    v2 = dpool.tile([P, n], f16, tag=f"v2{n}", bufs=2)
                    nc.vector.scalar_tensor_tensor(
                        out=v2[:], in0=v1[:], scalar=c2pp / c4pp, in1=xt[:],
                        op0=ALU.add, op1=ALU.mult)
                    dent = dpool.tile([P, n], f16, tag=f"v3{n}", bufs=3)
                    nc.vector.scalar_tensor_tensor(
                        out=dent[:], in0=v2[:], scalar=c1pp / c4pp, in1=xt[:],
                        op0=ALU.add, op1=ALU.mult)
                    sc = pg["sig_r"] * c4pp
                    emit_tail(b, n, r0, xt, dent, sc, rbias=pg["sig_r"])
                else:
                    dent = dpool.tile([P, n], f32, tag=f"d{n}",
                                      bufs=3 if n == full else 2)
                    nc.vector._custom_dve(
                        ops["KAT_DEN"],
                        out=dent[:], in0=xt[:], in1=ct[:, b:b + 1],
                        s0=c4pp, s1=c3pp, imm2=c2pp,
                    )
                    emit_tail(b, n, r0, xt, dent, pg["sig_r"])

            def pop_tail(ob):
                xt, rr = off_state.pop(ob)
                emit_tail(ob, RW, 0, xt, None, 0.0, rt=rr)

            xts = {}

            def emit_load(i):
                b, r0, n = pieces[i]
                if b in offload:
                    xt = xpool.tile([P, n], f16, tag="xo", bufs=5, name=f"xo{i}")
                else:
                    xt = xpool.tile([P, n], f16, tag=f"x{n}", name=f"xi{i}",
                                    bufs=5 if n == full else 4)
                nc.sync.dma_start(out=xt[:], in_=x[b * P:(b + 1) * P, r0:r0 + n])
                xts[i] = xt

            pieces = order_pieces(pieces, offload)
            for i in range(min(PREFETCH, len(pieces))):
                emit_load(i)
            pending = []          # (block, emit piece-index)
            for idx, (b, r0, n) in enumerate(pieces):
                if idx + PREFETCH < len(pieces):
                    emit_load(idx + PREFETCH)
                # pop deferred tails once their Pool chain is ~OFF_DELAY
                # pieces old, so they never trail the graduated drain pieces
                while pending and idx - pending[0][1] >= OFF_DELAY:
                    pop_tail(pending.pop(0)[0])
                if b in offload:
                    emit_off_den(b, n, r0, xts.pop(idx))
                    pending.append((b, idx))
                    continue
                emit_main(b, r0=r0, n=n, xt=xts.pop(idx))
            for ob, _ in pending:
                pop_tail(ob)
    nc.compile()
    return nc


def _kernel_pf(x, a, c):
    """Fast path driver. x:(B,L,D) f32. Returns (B,L,D) f32 or None."""
    from concourse import bass_utils

    params = derive_pf_params(a, c)
    if params is None:
        return None
    nc = _build_pf(params)

    Wg = D // G_FIXED
    lam_ch = np.repeat([p["lam"] for p in params], Wg)      # (D,)
    sig2_ch = np.repeat([p["sig2"] for p in params], Wg)    # (D,)

    # host: z = lam * x, transposed to [D, B*L], fp16
    x2 = np.asarray(x, np.float32).reshape(ROWS_ALL, D)
    zt = (x2.T * lam_ch[:, None].astype(np.float32)).astype(np.float16)

    coef_arr = np.zeros((P, 3 * NBLK), np.float32)
    for b in range(NBLK):
        g = b * P // Wg
        coef_arr[:, b] = params[g]["cden"][0]   # c1'' spilled via in1
        if params[g]["fac"] is not None:
            h1, k1, h2, k2 = params[g]["fac"]
            coef_arr[:, NBLK + 2 * b] = h1
            coef_arr[:, NBLK + 2 * b + 1] = h2

    in_maps = [
        {"x": np.ascontiguousarray(zt[:, core * RW:(core + 1) * RW]),
         "coef": coef_arr}
        for core in range(N_CORES)
    ]
    res = bass_utils.run_bass_kernel_spmd(nc, in_maps, core_ids=list(range(N_CORES)))

    out2 = np.empty((ROWS_ALL, D), np.float32)
    s32 = sig2_ch.astype(np.float32)[:, None]
    for core in range(N_CORES):
        ys = res.results[core]["y"]             # [D, RW] f16
        out2[core * RW:(core + 1) * RW, :] = (ys.astype(np.float32) * s32).T
    return out2.reshape(B, L, D)


# ---------------------------------------------------------------------------
# Fallback "opt" path (previous kernel): row-sharded f32, den/recip/num/mul.
# ---------------------------------------------------------------------------

def _build_opt_module(a, c, G):
    import concourse.bacc as bacc
    import concourse.mybir as mybir
    from concourse.tile import TileContext

    ops = _register_ops()
    f32 = mybir.dt.float32
    W = D // G
    imm = lambda v: mybir.ImmediateValue(dtype=mybir.dt.float32, value=v)

    nc = bacc.Bacc("TRN2", target_bir_lowering=False)
    x = nc.dram_tensor("x", (ROWS, D), f32, kind="ExternalInput")
    coef = nc.dram_tensor("coef", (P, G + 4), f32, kind="ExternalInput")
    y = nc.dram_tensor("y", (ROWS, D), f32, kind="ExternalOutput")

    with TileContext(nc) as tc:
        with tc.tile_pool(name="const", bufs=1) as cpool, \
             tc.tile_pool(name="xo", bufs=4) as xpool, \
             tc.tile_pool(name="work", bufs=3) as pool:
            ct = cpool.tile([P, G + 4], f32)
            nc.sync.dma_start(out=ct[:], in_=coef[:, :])
            for i in range(N_TILES):
                r0 = i * P
                xt = xpool.tile([P, D], f32, tag="x")
                nc.sync.dma_start(out=xt[:], in_=x[r0:r0 + P, :])
                dent = pool.tile([P, D], f32, tag="den")
                for g in range(G):
                    sl = slice(g * W, (g + 1) * W)
                    nc.vector._custom_dve(
                        ops["KAT_DEN"],
                        out=dent[:, sl], in0=xt[:, sl], in1=ct[:, g:g + 1],
                        s0=float(c[g, 4]), s1=float(c[g, 3]), imm2=float(c[g, 2]),
                    )
                rt = pool.tile([P, D], f32, tag="r")
                nc.scalar.add_instruction(
                    mybir.InstActivation(
                        name=nc.get_next_instruction_name(),
                        func=mybir.ActivationFunctionType.Reciprocal,
                        ins=[nc.scalar.lower_ap(dent[:]),
                             imm(0.0), imm(1.0), imm(0.0)],
                        outs=[nc.scalar.lower_ap(rt[:])],
                    )
                )
                qt = pool.tile([P, D], f32, tag="q")
                nc.vector._custom_dve(
                    ops["KAT_NUMQ"],
                    out=qt[:], in0=xt[:], in1=ct[:, G:G + 1],
                    s0=float(a[5]), s1=float(a[4]), imm2=float(a[3]),
                )
                mt = pool.tile([P, D], f32, tag="m")
                nc.vector._custom_dve(
                    ops["KAT_NUMM"],
                    out=mt[:], in0=qt[:], in1=xt[:],
                    s0=float(a[1]), s1=float(a[0]),
                )
                ot = xt
                nc.gpsimd.tensor_mul(ot[:], mt[:], rt[:])
                nc.sync.dma_start(out=y[r0:r0 + P, :], in_=ot[:])
    nc.compile()
    return nc


def _kernel_opt(x, a, c, G):
    from concourse import bass_utils

    nc = _build_opt_module(a, c, G)
    coef_arr = np.zeros((P, G + 4), np.float32)
    coef_arr[:, :G] = c[:, 1][None, :]
    coef_arr[:, G] = a[2]

    xr = np.asarray(x, np.float32).reshape(B, N_CORES, L_SH, D)
    in_maps = [
        {"x": np.ascontiguousarray(xr[:, core]).reshape(ROWS, D),
         "coef": coef_arr}
        for core in range(N_CORES)
    ]
    res = bass_utils.run_bass_kernel_spmd(nc, in_maps, core_ids=list(range(N_CORES)))
    out = np.empty((B, N_CORES, L_SH, D), np.float32)
    for core in range(N_CORES):
        out[:, core] = res.results[core]["y"].reshape(B, L_SH, D)
    return out.reshape(B, L, D)


def kernel(x, weight_numerator, weight_denominator, num_groups):
    x = np.ascontiguousarray(np.asarray(x, dtype=np.float32))
    a = np.asarray(weight_numerator, np.float32).reshape(-1)          # (6,)
    wd = np.asarray(weight_denominator, np.float32)                   # (G,4)
    G = int(num_groups)
    c = np.abs(np.concatenate([np.ones((G, 1), np.float32), wd], axis=1))

    if VARIANT == "pf" and G == G_FIXED and x.shape == (B, L, D):
        out = _kernel_pf(x, a, c)
        if out is not None:
            return out
    return _kernel_opt(x, a, c, G)
